# revision 1
# baseline (speedup 1.0000x reference)
"""Trainium2 Bass kernel for nn_Attention_Fusion_Bimodal_LNSpecific_Sum.

Bimodal deformable-attention encoder: conv1x1+GN on two modalities ->
concat tokens [800, 512] -> 6 encoder layers (deformable attention with
bilinear sampling, modality-specific LN, FFN) -> modality sum -> conv1x1+GN.

Sharding: pure data-parallel over batch (B=16 -> 2 batches per core x 8).

Device-side formulation highlights:
  - Bilinear sampling == dense sampling-matrix A[q, kv] @ V[kv, d] per
    (batch, head, level). A rows are separable tent products
    aw * relu(1-|y-iy|) * relu(1-|x-ix|) (exactly equivalent to the
    reference's corner gather + border clipping), built on the vector
    engine with stride-0 access-pattern expansion, then block-transposed
    via the DMA xbar (bf16) so TensorE can contract over kv.
  - Activations are feature-major [D, tok] for projections/LN; token-major
    tensors (coords, attention weights, value, A) come directly from matmul
    output orientation choices -- no layout transposes.
  - LN/GN partition-axis reductions run on TensorE via ones/indicator
    matmuls in f32 (PSUM accumulate), keeping norm stats at full precision.
"""

import contextlib

import numpy as np

import bass_rust
import concourse.bass as bass
import concourse.mybir as mybir
import concourse.tile as tile_mod
from bass_rust import ScopedClock
from concourse.tile import TileContext
from concourse.bass_utils import run_bass_kernel_spmd

dt = mybir.dt
AL = mybir.AluOpType
AF = mybir.ActivationFunctionType
AX = mybir.AxisListType

B, C, H, W = 16, 768, 20, 20
D, NL, Hd, L, P, Dff = 512, 6, 8, 2, 4, 1024
HW = H * W          # 400
T = 2 * HW          # 800
Dh = D // Hd        # 64
EPS = 1e-5
NCORES = 8
BPC = B // NCORES   # 2
GN_G = 32

F32 = dt.float32
BF16 = dt.bfloat16
NPBF = dt.np(BF16)

TOKC = [(0, 128), (128, 128), (256, 128), (384, 128),
        (512, 128), (640, 128), (768, 32)]
NSPL = [(0, 512), (512, 288)]       # psum-bank-aligned N splits for width 800


# ------------------------------------------------------- tile drain patch
def _patched_drain_and_barrier(self, tick_clock, wait_clock):
    nc = self.nc
    drain_inst = nc.sync.drain()
    wait_clock.add_sem_waits(
        drain_inst.ins, ScopedClock({None: tick_clock.global_clock})
    )
    si = drain_inst.ins.sync_info
    if si is not None and si.on_wait and len(si.on_wait) > 1:
        waits = list(si.on_wait)
        si.on_wait = waits[:1]
        for w in waits[1:]:
            n = nc.sync.nop()
            n.ins.sync_info = bass_rust.SyncInfo(on_wait=[w], on_update=[])
    nc.all_engine_barrier()
    assert self.sems is not None
    popped = nc._tile_sem_poison_stack.pop()
    assert popped is self._sem_poison
    nc.clear_and_free_semaphores(list(self.sems.allocated().values()))
    nc.all_engine_barrier()


tile_mod.TileContext._drain_and_barrier = _patched_drain_and_barrier


# ------------------------------------------------------------- host consts
def _sine_pos(h, w, d):
    nf = d // 2
    scale = 2.0 * np.pi
    ye = np.arange(1, h + 1, dtype=np.float32) / h * scale
    xe = np.arange(1, w + 1, dtype=np.float32) / w * scale
    dim_t = 10000.0 ** (2.0 * (np.arange(nf) // 2).astype(np.float32) / nf)

    def enc(e):
        p = e[:, None] / dim_t
        return np.stack(
            [np.sin(p[:, 0::2]), np.cos(p[:, 1::2])], -1
        ).reshape(e.shape[0], nf)

    py, px = enc(ye), enc(xe)
    pos = np.concatenate(
        [
            np.broadcast_to(py[:, None, :], (h, w, nf)),
            np.broadcast_to(px[None, :, :], (h, w, nf)),
        ],
        -1,
    )
    return pos.reshape(h * w, d).astype(np.float32)


def _gind(F):
    per = F // GN_G
    ind = np.zeros((F, GN_G), np.float32)
    for g in range(GN_G):
        ind[g * per:(g + 1) * per, g] = 1.0
    return ind


def host_prep(inputs):
    f32 = lambda a: np.ascontiguousarray(np.asarray(a, np.float32))
    bf16 = lambda a: np.ascontiguousarray(
        np.asarray(a, np.float32).astype(NPBF))

    pos = _sine_pos(H, W, D)
    le = np.asarray(inputs["level_embed"], np.float32)
    posf = np.concatenate([pos + le[0], pos + le[1]], 0)        # [800, 512]
    gx = (np.arange(W, dtype=np.float32) + 0.5) / W
    gy = (np.arange(H, dtype=np.float32) + 0.5) / H
    X, Y = np.meshgrid(gx, gy)
    ref1 = np.stack([X, Y], -1).reshape(HW, 2)
    ref = np.concatenate([ref1, ref1], 0)                       # [800,2] (x,y)
    refg = np.empty((T, 2), np.float32)
    refg[:, 0] = ref[:, 0] * W - 0.5
    refg[:, 1] = ref[:, 1] * H - 0.5

    d = {}
    d["avT"] = bf16(np.asarray(inputs["av_w"]).T)
    d["aiT"] = bf16(np.asarray(inputs["ai_w"]).T)
    d["asT"] = bf16(np.asarray(inputs["as_w"]).T)
    d["convb_v"] = f32(np.asarray(inputs["av_b"])[:, None])
    d["convb_i"] = f32(np.asarray(inputs["ai_b"])[:, None])
    d["convb_s"] = f32(np.asarray(inputs["as_b"])[:, None])
    d["gng_v"] = f32(np.asarray(inputs["av_g"])[:, None])
    d["gnb_v"] = f32(np.asarray(inputs["av_be"])[:, None])
    d["gng_i"] = f32(np.asarray(inputs["ai_g"])[:, None])
    d["gnb_i"] = f32(np.asarray(inputs["ai_be"])[:, None])
    d["gng_s"] = f32(np.asarray(inputs["as_g"])[:, None])
    d["gnb_s"] = f32(np.asarray(inputs["as_be"])[:, None])
    d["ind512"] = f32(_gind(512))
    d["exp512"] = f32(_gind(512).T)
    d["ind768"] = f32(_gind(768))
    d["exp768"] = f32(_gind(768).T)
    d["ones128f"] = f32(np.ones((128, 1)))
    d["ones1f"] = f32(np.ones((1, 128)))
    d["ones1x128"] = bf16(np.ones((1, 128)))
    d["iota20"] = f32(np.broadcast_to(
        np.arange(20, dtype=np.float32)[None, :], (128, 20)))

    off_w = np.asarray(inputs["off_w"], np.float32)
    off_b = np.asarray(inputs["off_b"], np.float32)
    aw_w = np.asarray(inputs["aw_w"], np.float32)
    aw_b = np.asarray(inputs["aw_b"], np.float32)
    d["off_w"] = bf16(off_w)
    d["aw_w"] = bf16(aw_w)
    coord_add = np.einsum("td,ldj->ltj", posf, off_w) + off_b[:, None, :]
    ca = coord_add.reshape(NL, T, Hd * L * P, 2)
    ca[..., 0] += refg[None, :, None, 0]
    ca[..., 1] += refg[None, :, None, 1]
    d["coordAdd"] = f32(coord_add)
    d["awAdd"] = f32(np.einsum("td,ldj->ltj", posf, aw_w) + aw_b[:, None, :])
    d["vp_w"] = bf16(np.asarray(inputs["vp_w"]))
    d["vp_brow"] = bf16(np.asarray(inputs["vp_b"])[:, None, :])
    d["op_w"] = bf16(np.asarray(inputs["op_w"]))
    d["op_b"] = f32(np.asarray(inputs["op_b"])[:, :, None])
    d["ffn1_w"] = bf16(np.asarray(inputs["ffn1_w"]))
    d["ffn1_b"] = f32(np.asarray(inputs["ffn1_b"])[:, :, None])
    d["ffn2_w"] = bf16(np.asarray(inputs["ffn2_w"]))
    d["ffn2_b"] = f32(np.asarray(inputs["ffn2_b"])[:, :, None])
    for nm in ("ln1", "ln2"):
        g = np.asarray(inputs[f"{nm}_g"], np.float32)
        b = np.asarray(inputs[f"{nm}_b"], np.float32)
        d[nm] = f32(np.stack([g[:, 0], b[:, 0], g[:, 1], b[:, 1]], -1))
    return d


DRAM_SPECS = {
    "avT": ([C, D], BF16), "aiT": ([C, D], BF16), "asT": ([D, C], BF16),
    "convb_v": ([D, 1], F32), "convb_i": ([D, 1], F32),
    "convb_s": ([C, 1], F32),
    "gng_v": ([D, 1], F32), "gnb_v": ([D, 1], F32),
    "gng_i": ([D, 1], F32), "gnb_i": ([D, 1], F32),
    "gng_s": ([C, 1], F32), "gnb_s": ([C, 1], F32),
    "ind512": ([D, GN_G], F32), "exp512": ([GN_G, D], F32),
    "ind768": ([C, GN_G], F32), "exp768": ([GN_G, C], F32),
    "ones128f": ([128, 1], F32), "ones1f": ([1, 128], F32),
    "ones1x128": ([1, 128], BF16),
    "iota20": ([128, 20], F32),
    "off_w": ([NL, D, 128], BF16), "aw_w": ([NL, D, 64], BF16),
    "coordAdd": ([NL, T, 128], F32), "awAdd": ([NL, T, 64], F32),
    "vp_w": ([NL, D, D], BF16), "vp_brow": ([NL, 1, D], BF16),
    "op_w": ([NL, D, D], BF16), "op_b": ([NL, D, 1], F32),
    "ffn1_w": ([NL, D, Dff], BF16), "ffn1_b": ([NL, Dff, 1], F32),
    "ffn2_w": ([NL, Dff, D], BF16), "ffn2_b": ([NL, D, 1], F32),
    "ln1": ([NL, D, 4], F32), "ln2": ([NL, D, 4], F32),
}

STREAMED = {"off_w", "aw_w", "coordAdd", "awAdd", "vp_w", "vp_brow", "op_w",
            "op_b", "ffn1_w", "ffn1_b", "ffn2_w", "ffn2_b", "ln1", "ln2"}

# per-tag buffer counts for streamed layer weights (single-buffered)
W_BUFS = {"off_w": 4, "aw_w": 4, "vp_w": 4, "vp_brow": 1, "op_w": 4,
          "ffn1_w": 4, "ffn1_b": 8, "ffn2_w": 8, "ffn2_b": 4,
          "op_b": 4, "ln1": 4, "ln2": 4, "coordAdd": 7, "awAdd": 7}


def _split_multiwaits(nc, max_waits=1):
    """walrus rejects instructions carrying more than one sync-wait; hoist
    extra waits onto same-engine nops placed before the instruction."""
    for f in nc.m.functions:
        for blk in f.blocks:
            out = []
            changed = False
            for inst in blk.instructions:
                si = inst.sync_info
                if si is not None and si.on_wait and \
                        len(si.on_wait) > max_waits:
                    waits = list(si.on_wait)
                    for j, w in enumerate(waits[:-max_waits]):
                        n = mybir.InstNoOp(name=f"{inst.name}_w{j}", ins=[],
                                           outs=[])
                        n.engine = inst.engine
                        n.sync_info = bass_rust.SyncInfo(on_wait=[w],
                                                         on_update=[])
                        out.append(n)
                    si.on_wait = waits[-max_waits:]
                    changed = True
                out.append(inst)
            if changed:
                blk.instructions = out


class Ctx:
    pass


def build_kernel(num_layers=NL, num_batches=BPC, taps=()):
    nc = bass.Bass("TRN2", target_bir_lowering=False, debug=False,
                   num_devices=NCORES)
    g = Ctx()
    g.nc = nc
    g.NLs = num_layers
    g.BPCs = num_batches
    g.taps = set(taps)
    g.tap_d = {}

    g.dx_v = nc.dram_tensor("xv", [BPC, C, HW], F32, kind="ExternalInput")
    g.dx_i = nc.dram_tensor("xi", [BPC, C, HW], F32, kind="ExternalInput")
    g.out_d = nc.dram_tensor("out", [BPC, C, HW], F32, kind="ExternalOutput")
    g.dram = {nm: nc.dram_tensor(nm, shp, ty, kind="ExternalInput")
              for nm, (shp, ty) in DRAM_SPECS.items()}

    with TileContext(nc) as tc:
        g.tc = tc
        with contextlib.ExitStack() as ctx:
            _body(ctx, g)
    _split_multiwaits(nc)
    return nc


def _tap_fm(g, name, tiles, rows, cols, cast=True):
    """Dump feature-major tiles (list of [128, cols]) to a dram tap."""
    if name not in g.taps:
        return
    d = g.nc.dram_tensor(f"tap_{name}", [rows, cols], F32,
                         kind="ExternalOutput")
    g.tap_d[name] = d
    for k, tl in enumerate(tiles):
        r = min(128, rows - k * 128)
        if tl.dtype != F32 and cast:
            tf = g.scr.tile([128, cols], F32, tag="tapf")
            g.nc.vector.tensor_copy(tf[:r, :], tl[:r, :cols])
            g.dma(d[k * 128:k * 128 + r, :], tf[:r, :])
        else:
            g.dma(d[k * 128:k * 128 + r, :], tl[:r, :cols])


def _body(ctx, g):
    nc, tc = g.nc, g.tc
    pool = lambda name, bufs, **kw: ctx.enter_context(
        tc.tile_pool(name=name, bufs=bufs, **kw))
    g.cpool = pool("consts", 1)
    g.wpool = pool("weights", 1)
    g.state = pool("state", 1)
    g.scr = pool("scratch", 2)
    g.mps = pool("mpsum", 2, space="PSUM")
    g.sps = pool("spsum", 2, space="PSUM")
    g.dma = nc.sync.dma_start

    # ---------------- consts to SBUF ----------------
    cw = {}
    for nm, (shp, ty) in DRAM_SPECS.items():
        if nm in STREAMED:
            continue
        t = g.dram[nm]
        KX = shp[0]
        if KX <= 128:
            tl = g.cpool.tile([KX, shp[1]], ty, name=f"c_{nm}")
            g.dma(tl[:, :], t[:, :])
            cw[nm] = tl
        else:
            tiles = []
            for k0 in range(0, KX, 128):
                tl = g.cpool.tile([128, shp[1]], ty, name=f"c_{nm}_{k0}")
                g.dma(tl[:, :], t[k0:k0 + 128, :])
                tiles.append(tl)
            cw[nm] = tiles
    g.cw = cw

    # ---------------- persistent state ----------------
    g.srcF = [[g.state.tile([128, T], F32, name=f"srcF_{b}_{k}")
               for k in range(4)] for b in range(g.BPCs)]
    g.srcB = [[g.state.tile([128, T], BF16, name=f"srcB_{b}_{k}")
               for k in range(4)] for b in range(g.BPCs)]

    for b in range(g.BPCs):
        _input_stage(g, b)
    for l in range(g.NLs):
        lw = _load_layer_weights(g, l)
        for b in range(g.BPCs):
            _layer(g, l, b, lw)
    for b in range(g.BPCs):
        _output_stage(g, b)


def _psum_big(g):
    return g.mps.tile([128, 1024], F32, tag="big", name="psbig")


def _mm_acc(g, ps, pairs, n_total, m_rows=None, n_split=512):
    """ps[:mr, :n_total] = sum_k lhsT_k.T @ rhs_k  (bank-aligned N splits)."""
    mr = m_rows if m_rows is not None else ps.shape[0]
    for n0 in range(0, n_total, n_split):
        n1 = min(n0 + n_split, n_total)
        for i, (lt, rh) in enumerate(pairs):
            g.nc.tensor.matmul(
                ps[:mr, n0:n1], lt, rh[:, n0:n1],
                start=(i == 0), stop=(i == len(pairs) - 1),
            )


def _load_layer_weights(g, l):
    lw = {}

    def ld(name):
        t = g.dram[name]
        KX, MX = t.shape[1], t.shape[2]
        tiles = []
        for k0 in range(0, KX, 128):
            kk = min(128, KX - k0)
            tl = g.wpool.tile([128, MX], t.dtype, tag=f"w_{name}",
                              bufs=W_BUFS[name], name=f"{name}_l{l}_{k0}")
            g.dma(tl[:kk, :], t[l, k0:k0 + kk, :])
            tiles.append(tl)
        return tiles

    for nm in ("off_w", "aw_w", "vp_w", "vp_brow", "op_w", "ffn1_w",
               "ffn2_w", "op_b", "ffn1_b", "ffn2_b", "ln1", "ln2"):
        lw[nm] = ld(nm)
    lw["coordAdd"] = []
    lw["awAdd"] = []
    for (t0, sz) in TOKC:
        ca = g.wpool.tile([128, 128], F32, tag="w_coordAdd", bufs=7,
                          name=f"coordAdd_l{l}_{t0}")
        g.dma(ca[:sz, :], g.dram["coordAdd"][l, t0:t0 + sz, :])
        lw["coordAdd"].append(ca)
        aa = g.wpool.tile([128, 64], F32, tag="w_awAdd", bufs=7,
                          name=f"awAdd_l{l}_{t0}")
        g.dma(aa[:sz, :], g.dram["awAdd"][l, t0:t0 + sz, :])
        lw["awAdd"].append(aa)
    return lw


def _input_stage(g, b):
    nc, cw = g.nc, g.cw
    for (src_d, wT, bias, gg, gb) in (
        (g.dx_v, cw["avT"], cw["convb_v"], cw["gng_v"], cw["gnb_v"]),
        (g.dx_i, cw["aiT"], cw["convb_i"], cw["gng_i"], cw["gnb_i"]),
    ):
        half = 0 if src_d is g.dx_v else HW
        xb = []
        for ki, k0 in enumerate(range(0, C, 128)):
            xf = g.scr.tile([128, HW], F32, tag="ln_sq", bufs=2, name="xf")
            g.dma(xf[:, :], src_d[b, k0:k0 + 128, :])
            xc = g.scr.tile([128, HW], BF16, tag="vtok", bufs=8, name="xc")
            nc.vector.tensor_copy(xc[:, :], xf[:, :])
            xb.append(xc)
        for m in range(4):
            ps = _psum_big(g)
            _mm_acc(g, ps, [(wT[k][:, m * 128:(m + 1) * 128], xb[k])
                            for k in range(6)], n_total=HW)
            nc.scalar.activation(g.srcF[b][m][:, half:half + HW],
                                 ps[:, :HW], AF.Identity,
                                 bias=bias[m][:, :],
                                 scale=1.0)
        _groupnorm(g, [g.srcF[b][k] for k in range(4)], half, HW,
                   cw["ind512"], cw["exp512"], 512, gg, gb,
                   out_bf=[(g.srcB[b][k], half) for k in range(4)])
    _tap_fm(g, f"src0_{b}", g.srcF[b], D, T, cast=False)


def _groupnorm(g, featF, col0, ncols, ind, exp, F, gcol, bcol, out_bf):
    """In-place f32 GroupNorm on feature-major tiles over columns
    [col0:col0+ncols]; optional bf16 shadow writes."""
    nc = g.nc
    nk = F // 128
    per = F // GN_G
    inv = 1.0 / (per * ncols)
    indl = ind if isinstance(ind, list) else [ind]

    ps = _psum_big(g)          # use [32, 2*ncols] view
    for k in range(nk):
        sq = g.scr.tile([128, ncols], F32, tag="gn_sq", bufs=2, name="gn_sq")
        nc.scalar.activation(sq[:, :], featF[k][:, col0:col0 + ncols],
                             AF.Square)
        it = indl[k] if len(indl) > 1 else indl[0]
        nc.tensor.matmul(ps[:GN_G, 0:ncols], it[:, :],
                         featF[k][:, col0:col0 + ncols],
                         start=(k == 0), stop=(k == nk - 1))
        nc.tensor.matmul(ps[:GN_G, 512:512 + ncols], it[:, :], sq[:, :],
                         start=(k == 0), stop=(k == nk - 1))
    red = g.scr.tile([GN_G, 2], F32, tag="gn_red", bufs=2)
    nc.vector.tensor_reduce(red[:, 0:1], ps[:GN_G, 0:ncols], AX.X, AL.add)
    nc.vector.tensor_reduce(red[:, 1:2], ps[:GN_G, 512:512 + ncols], AX.X,
                            AL.add)
    st = g.scr.tile([GN_G, 4], F32, tag="gn_st", bufs=2)
    # st0 = mean, st1 = E[x^2], st2 = var, st3 = rsqrt(var+eps)
    nc.vector.tensor_scalar(st[:, 0:2], red[:, 0:2], inv, None, AL.mult)
    nc.vector.tensor_tensor(st[:, 2:3], st[:, 0:1], st[:, 0:1], AL.mult)
    nc.vector.tensor_tensor(st[:, 2:3], st[:, 1:2], st[:, 2:3], AL.subtract)
    nc.vector.tensor_scalar(st[:, 2:3], st[:, 2:3], float(EPS), None, AL.add)
    nc.scalar.activation(st[:, 3:4], st[:, 2:3], AF.Sqrt)
    nc.vector.reciprocal(st[:, 3:4], st[:, 3:4])
    expl = exp if isinstance(exp, list) else [exp]
    for k in range(nk):
        et = (expl[0][:, k * 128:(k + 1) * 128] if len(expl) == 1
              else expl[k][:, :])
        eps_ = _psum_big(g)
        nc.tensor.matmul(eps_[:, 0:1], et, st[:, 0:1], start=True, stop=True)
        nc.tensor.matmul(eps_[:, 1:2], et, st[:, 3:4], start=True, stop=True)
        sc = g.scr.tile([128, 2], F32, tag="gn_sc", bufs=2)
        nc.vector.tensor_tensor(sc[:, 0:1], eps_[:, 1:2],
                                gcol[k][:, :], AL.mult)
        nc.vector.tensor_tensor(sc[:, 1:2], eps_[:, 0:1], sc[:, 0:1],
                                AL.mult)
        nc.vector.tensor_tensor(sc[:, 1:2], bcol[k][:, :],
                                sc[:, 1:2], AL.subtract)
        nc.vector.tensor_scalar(featF[k][:, col0:col0 + ncols],
                                featF[k][:, col0:col0 + ncols],
                                sc[:, 0:1], sc[:, 1:2], AL.mult, AL.add)
        if out_bf is not None:
            bt, boff = out_bf[k]
            nc.vector.tensor_copy(bt[:, boff:boff + ncols],
                                  featF[k][:, col0:col0 + ncols])


def _ln_spec(g, b, hF, lncols):
    """Modality-specific LayerNorm over features (partition axis), feature-
    major. hF: 4 f32 tiles [128, 800]. Writes srcF (f32) + srcB (bf16)."""
    nc, cw = g.nc, g.cw
    ones = cw["ones128f"]
    inv = 1.0 / D
    ps1 = _psum_big(g)         # row 0 = sum
    ps2 = _psum_big(g)         # row 0 = sumsq
    for k in range(4):
        sq = g.scr.tile([128, T], F32, tag="ln_sq", bufs=2, name="ln_sq")
        nc.scalar.activation(sq[:, :], hF[k][:, :], AF.Square)
        for (n0, nn_) in NSPL:
            nc.tensor.matmul(ps1[0:1, n0:n0 + nn_], ones[:, :],
                             hF[k][:, n0:n0 + nn_],
                             start=(k == 0), stop=(k == 3))
            nc.tensor.matmul(ps2[0:1, n0:n0 + nn_], ones[:, :],
                             sq[:, n0:n0 + nn_],
                             start=(k == 0), stop=(k == 3))
    stm = g.scr.tile([1, T], F32, tag="ln_stm", bufs=2, name="ln_stm")
    sts = g.scr.tile([1, T], F32, tag="ln_sts", bufs=2, name="ln_sts")
    nc.vector.tensor_scalar(stm[:, :], ps1[0:1, 0:T], inv, None, AL.mult)
    nc.vector.tensor_scalar(sts[:, :], ps2[0:1, 0:T], inv, None, AL.mult)
    v = g.scr.tile([1, T], F32, tag="ln_v", bufs=2, name="ln_v")
    nc.vector.tensor_tensor(v[:, :], stm[:, :], stm[:, :], AL.mult)
    nc.vector.tensor_tensor(v[:, :], sts[:, :], v[:, :], AL.subtract)
    nc.vector.tensor_scalar(v[:, :], v[:, :], float(EPS), None, AL.add)
    nc.scalar.activation(sts[:, :], v[:, :], AF.Sqrt)
    nc.vector.reciprocal(sts[:, :], sts[:, :])
    # broadcast m/s rows to [128, T] via K=1 f32 matmuls (kept in PSUM)
    psm = _psum_big(g)
    pss = _psum_big(g)
    for psr, row in ((psm, stm), (pss, sts)):
        for (n0, nn_) in NSPL:
            nc.tensor.matmul(psr[:, n0:n0 + nn_], cw["ones1f"][:, :],
                             row[0:1, n0:n0 + nn_],
                             start=True, stop=True)
    for k in range(4):
        # in-place: h = (h - m) * s
        nc.vector.tensor_tensor(hF[k][:, :], hF[k][:, :], psm[:, 0:T],
                                AL.subtract)
        nc.vector.tensor_tensor(hF[k][:, :], hF[k][:, :], pss[:, 0:T],
                                AL.mult)
        lc = lncols[k]
        nc.scalar.activation(g.srcF[b][k][:, 0:HW], hF[k][:, 0:HW],
                             AF.Identity, bias=lc[:, 1:2], scale=lc[:, 0:1])
        nc.scalar.activation(g.srcF[b][k][:, HW:T], hF[k][:, HW:T],
                             AF.Identity, bias=lc[:, 3:4], scale=lc[:, 2:3])
        nc.vector.tensor_copy(g.srcB[b][k][:, :], g.srcF[b][k][:, :])


def _layer(g, l, b, lw):
    nc, cw = g.nc, g.cw
    srcB = g.srcB[b]

    # ---------------- coords + attention weights (token-major) ----------
    coords, awt = [], []
    for ci, (t0, sz) in enumerate(TOKC):
        ps = _psum_big(g)
        _mm_acc(g, ps, [(srcB[k][:, t0:t0 + sz], w)
                        for k, w in enumerate(lw["off_w"])],
                n_total=128, m_rows=sz)
        ct = g.scr.tile([128, 128], F32, tag="coords", bufs=7,
                        name=f"co_{l}_{b}_{ci}")
        nc.vector.tensor_tensor(ct[:sz, :], ps[:sz, 0:128],
                                lw["coordAdd"][ci][:sz, :], AL.add)
        coords.append(ct)

        ps2 = _psum_big(g)
        _mm_acc(g, ps2, [(srcB[k][:, t0:t0 + sz], w)
                         for k, w in enumerate(lw["aw_w"])],
                n_total=64, m_rows=sz)
        at = g.scr.tile([128, 64], F32, tag="awt", bufs=7,
                        name=f"aw_{l}_{b}_{ci}")
        nc.vector.tensor_tensor(at[:sz, :], ps2[:sz, 0:64],
                                lw["awAdd"][ci][:sz, :], AL.add)
        at3 = at[:sz, :].rearrange("q (h e) -> q h e", e=8)
        mx = g.scr.tile([128, Hd], F32, tag="aw_mx", bufs=2)
        nc.vector.tensor_reduce(mx[:sz, :], at3, AX.X, AL.max)
        nc.vector.tensor_tensor(
            at3, at3, mx[:sz, :].unsqueeze(-1).broadcast_to([sz, Hd, 8]),
            AL.subtract)
        nc.scalar.activation(at[:sz, :], at[:sz, :], AF.Exp)
        sm = g.scr.tile([128, Hd], F32, tag="aw_sm", bufs=2)
        nc.vector.tensor_reduce(sm[:sz, :], at3, AX.X, AL.add)
        nc.vector.reciprocal(sm[:sz, :], sm[:sz, :])
        nc.vector.tensor_tensor(
            at3, at3, sm[:sz, :].unsqueeze(-1).broadcast_to([sz, Hd, 8]),
            AL.mult)
        awt.append(at)

    # ---------------- value (token-major, zero-padded rows) -------------
    vtok = []
    for i8 in range(8):
        lv, j = divmod(i8, 4)
        tbase = lv * HW + j * 128
        sz = min(128, (lv + 1) * HW - tbase)
        vt = g.scr.tile([128, D], BF16, tag="vtok", bufs=8,
                        name=f"vt_{l}_{b}_{i8}")
        ps = _psum_big(g)
        for (n0, nn_) in ((0, 512),):
            for k, w in enumerate(lw["vp_w"]):
                nc.tensor.matmul(ps[:sz, n0:n0 + nn_],
                                 srcB[k][:, tbase:tbase + sz],
                                 w[:, n0:n0 + nn_], start=(k == 0),
                                 stop=False)
            nc.tensor.matmul(ps[:sz, n0:n0 + nn_],
                             cw["ones1x128"][:, :sz],
                             lw["vp_brow"][0][0:1, n0:n0 + nn_],
                             start=False, stop=True)
        if sz < 128:
            nc.gpsimd.memset(vt[:, :], 0.0)
        nc.scalar.activation(vt[:sz, :], ps[:sz, 0:D], AF.Copy)
        vtok.append(vt)

    # ---------------- per-head-pair tents; per-head A build + sample ----
    attnT = [g.scr.tile([128, T], BF16, tag="attnT", bufs=4,
                        name=f"attnT_{l}_{b}_{k}") for k in range(4)]
    iota = cw["iota20"]
    for hp in range(Hd // 2):          # head pairs
        # tents for heads (2*hp, 2*hp+1): [q, 16, 20] per axis
        tyP, txP = [], []
        for ci, (t0, sz) in enumerate(TOKC):
            ya = g.scr.tile([128, 16 * 20], BF16, tag="typair", bufs=7,
                            name=f"ty_{l}_{b}_{hp}_{ci}")
            xa = g.scr.tile([128, 16 * 20], BF16, tag="txpair", bufs=7,
                            name=f"tx_{l}_{b}_{hp}_{ci}")
            ct = coords[ci]
            for (dst, coff) in ((ya, 1), (xa, 0)):
                d3 = dst[:sz, :].rearrange("q (j i) -> q j i", i=20)
                csl = ct[:sz, 32 * hp + coff:32 * (hp + 1):2] \
                    .unsqueeze(-1).broadcast_to([sz, 16, 20])
                io = iota[:sz, :].unsqueeze(1).broadcast_to([sz, 16, 20])
                nc.vector.tensor_tensor(d3, io, csl, AL.subtract)
                nc.scalar.activation(dst[:sz, :], dst[:sz, :], AF.Abs)
                nc.scalar.activation(dst[:sz, :], dst[:sz, :], AF.Relu,
                                     bias=1.0, scale=-1.0)
            tyP.append(ya)
            txP.append(xa)
        for hh in range(2):
            h = 2 * hp + hh
            sps = g.sps.tile([64, T], F32, tag="samp", bufs=2, name="samp")
            first_mm = True
            for lv in range(L):
                at4 = [g.scr.tile([128, T], BF16, tag=f"at_{kc}", bufs=2,
                                  name=f"at_{l}_{b}_{h}_{lv}_{kc}")
                       for kc in range(4)]
                for ci, (t0, sz) in enumerate(TOKC):
                    grp = hh * 8 + lv * 4        # within pair tents
                    gcol = h * 8 + lv * 4        # within awt
                    a_t = g.scr.tile([128, 512], BF16, tag="a_t", bufs=3,
                                     name=f"a_{l}_{b}_{h}_{lv}_{ci}")
                    pt = g.scr.tile([128, 400], BF16, tag="p_t", bufs=3,
                                    name=f"p_{l}_{b}_{h}_{lv}_{ci}")
                    for p in range(P):
                        ty = tyP[ci][:sz, (grp + p) * 20:(grp + p + 1) * 20]
                        tx = txP[ci][:sz, (grp + p) * 20:(grp + p + 1) * 20]
                        tyv = ty.unsqueeze(-1).broadcast_to([sz, 20, 20])
                        txv = tx.unsqueeze(1).broadcast_to([sz, 20, 20])
                        awcol = awt[ci][:sz, gcol + p:gcol + p + 1]
                        dst = (a_t[:sz, 0:400] if p == 0 else pt[:sz, :]) \
                            .rearrange("q (a c) -> q a c", c=20)
                        nc.vector.scalar_tensor_tensor(
                            dst, tyv, awcol, txv, AL.mult, AL.mult)
                        if p > 0:
                            nc.vector.tensor_tensor(
                                a_t[:sz, 0:400], a_t[:sz, 0:400],
                                pt[:sz, :], AL.add)
                    nc.gpsimd.memset(a_t[:sz, 400:512], 0.0)
                    for kc in range(4):
                        nc.sync.dma_start_transpose(
                            at4[kc][:, t0:t0 + sz],
                            a_t[:sz, kc * 128:(kc + 1) * 128])
                for kc in range(4):
                    vsl = vtok[lv * 4 + kc][:, h * Dh:(h + 1) * Dh]
                    for si, (n0, nn_) in enumerate(NSPL):
                        last = (lv == L - 1 and kc == 3)
                        nc.tensor.matmul(sps[:, n0:n0 + nn_], vsl,
                                         at4[kc][:, n0:n0 + nn_],
                                         start=first_mm, stop=last)
                    first_mm = False
            kc_, ro = divmod(h * Dh, 128)
            nc.scalar.activation(attnT[kc_][ro:ro + Dh, :], sps[:, :],
                                 AF.Copy)

    _tap_fm(g, f"attnT_{l}_{b}", attnT, D, T)

    # ---------------- output projection + residual + LN1 ----------------
    hF = []
    for m in range(4):
        ps = _psum_big(g)
        _mm_acc(g, ps, [(w[:, m * 128:(m + 1) * 128], attnT[k])
                        for k, w in enumerate(lw["op_w"])], n_total=T)
        h_ = g.scr.tile([128, T], F32, tag="hF", bufs=4,
                        name=f"h1_{l}_{b}_{m}")
        nc.vector.scalar_tensor_tensor(
            h_[:, :], ps[:, 0:T], lw["op_b"][m][:, 0:1],
            g.srcF[b][m][:, :], AL.add, AL.add)
        hF.append(h_)
    _ln_spec(g, b, hF, lw["ln1"])
    _tap_fm(g, f"src1_{l}_{b}", g.srcF[b], D, T, cast=False)

    # ---------------- FFN + residual + LN2 ----------------
    ffb = []
    for m in range(8):
        ps = _psum_big(g)
        _mm_acc(g, ps, [(w[:, m * 128:(m + 1) * 128], g.srcB[b][k])
                        for k, w in enumerate(lw["ffn1_w"])], n_total=T)
        f_ = g.scr.tile([128, T], BF16, tag="ffb", bufs=8,
                        name=f"ff_{l}_{b}_{m}")
        nc.scalar.activation(f_[:, :], ps[:, 0:T], AF.Relu,
                             bias=lw["ffn1_b"][m][:, 0:1], scale=1.0)
        ffb.append(f_)
    hF2 = []
    for m in range(4):
        ps = _psum_big(g)
        _mm_acc(g, ps, [(w[:, m * 128:(m + 1) * 128], ffb[k])
                        for k, w in enumerate(lw["ffn2_w"])], n_total=T)
        h_ = g.scr.tile([128, T], F32, tag="hF", bufs=4,
                        name=f"h2_{l}_{b}_{m}")
        nc.vector.scalar_tensor_tensor(
            h_[:, :], ps[:, 0:T], lw["ffn2_b"][m][:, 0:1],
            g.srcF[b][m][:, :], AL.add, AL.add)
        hF2.append(h_)
    _ln_spec(g, b, hF2, lw["ln2"])
    _tap_fm(g, f"src2_{l}_{b}", g.srcF[b], D, T, cast=False)


def _output_stage(g, b):
    nc, cw = g.nc, g.cw
    ob = []
    for k in range(4):
        s = g.scr.tile([128, HW], F32, tag="ln_sq", bufs=2, name="os")
        nc.vector.tensor_tensor(s[:, :], g.srcF[b][k][:, 0:HW],
                                g.srcF[b][k][:, HW:T], AL.add)
        sb = g.scr.tile([128, HW], BF16, tag="vtok", bufs=8, name="osb")
        nc.vector.tensor_copy(sb[:, :], s[:, :])
        ob.append(sb)
    outF = []
    for m in range(6):
        ps = _psum_big(g)
        _mm_acc(g, ps, [(cw["asT"][k][:, m * 128:(m + 1) * 128], ob[k])
                        for k in range(4)], n_total=HW)
        of = g.scr.tile([128, HW], F32, tag="of", bufs=6,
                        name=f"outF_{b}_{m}")
        nc.scalar.activation(of[:, :], ps[:, :HW], AF.Identity,
                             bias=cw["convb_s"][m][:, :],
                             scale=1.0)
        outF.append(of)
    _groupnorm(g, outF, 0, HW, cw["ind768"], cw["exp768"], C,
               cw["gng_s"], cw["gnb_s"], out_bf=None)
    for k in range(6):
        g.dma(g.out_d[b, k * 128:(k + 1) * 128, :], outF[k][:, :])


# ---------------------------------------------------------------- entry
_CACHED = {}


def _get_nc():
    if "nc" not in _CACHED:
        _CACHED["nc"] = build_kernel()
    return _CACHED["nc"]


def kernel(**inputs):
    consts = host_prep(inputs)
    xv = np.asarray(inputs["input_v"], np.float32).reshape(B, C, HW)
    xi = np.asarray(inputs["input_i"], np.float32).reshape(B, C, HW)
    nc = _get_nc()
    in_maps = []
    for c in range(NCORES):
        m = dict(consts)
        m["xv"] = np.ascontiguousarray(xv[c * BPC:(c + 1) * BPC])
        m["xi"] = np.ascontiguousarray(xi[c * BPC:(c + 1) * BPC])
        in_maps.append(m)
    res = run_bass_kernel_spmd(nc, in_maps, core_ids=list(range(NCORES)))
    out = np.concatenate([res.results[c]["out"] for c in range(NCORES)], 0)
    return out.reshape(B, C, H, W).astype(np.float32)



# revision 20
# speedup vs baseline: 1.2202x; 1.2202x over previous
"""Trainium2 Bass kernel for nn_Attention_Fusion_Bimodal_LNSpecific_Sum.

Bimodal deformable-attention encoder: conv1x1+GN on two modalities ->
concat tokens [800, 512] -> 6 encoder layers (deformable attention with
bilinear sampling, modality-specific LN, FFN) -> modality sum -> conv1x1+GN.

Sharding: pure data-parallel over batch (B=16 -> 2 batches per core x 8).

Device-side formulation highlights:
  - Bilinear sampling == dense sampling-matrix A[q, kv] @ V[kv, d] per
    (batch, head, level). A rows are separable tent products
    aw * relu(1-|y-iy|) * relu(1-|x-ix|) (exactly equivalent to the
    reference's corner gather + border clipping), built on the vector
    engine with stride-0 access-pattern expansion, then block-transposed
    via the DMA xbar (bf16) so TensorE can contract over kv.
  - Activations are feature-major [D, tok] for projections/LN; token-major
    tensors (coords, attention weights, value, A) come directly from matmul
    output orientation choices -- no layout transposes.
  - LN/GN partition-axis reductions run on TensorE via ones/indicator
    matmuls in f32 (PSUM accumulate), keeping norm stats at full precision.
"""

import contextlib

import numpy as np

import bass_rust
import concourse.bass as bass
import concourse.mybir as mybir
import concourse.tile as tile_mod
from bass_rust import ScopedClock
from concourse.tile import TileContext
from concourse.bass_utils import run_bass_kernel_spmd

dt = mybir.dt
AL = mybir.AluOpType
AF = mybir.ActivationFunctionType
AX = mybir.AxisListType

B, C, H, W = 16, 768, 20, 20
D, NL, Hd, L, P, Dff = 512, 6, 8, 2, 4, 1024
HW = H * W          # 400
T = 2 * HW          # 800
Dh = D // Hd        # 64
EPS = 1e-5
NCORES = 8
BPC = B // NCORES   # 2
GN_G = 32

F32 = dt.float32
BF16 = dt.bfloat16
NPBF = dt.np(BF16)

TOKC = [(0, 128), (128, 128), (256, 128), (384, 128),
        (512, 128), (640, 128), (768, 32)]
NSPL = [(0, 512), (512, 288)]       # psum-bank-aligned N splits for width 800


# ------------------------------------------------------- tile drain patch
def _patched_drain_and_barrier(self, tick_clock, wait_clock):
    nc = self.nc
    drain_inst = nc.sync.drain()
    wait_clock.add_sem_waits(
        drain_inst.ins, ScopedClock({None: tick_clock.global_clock})
    )
    si = drain_inst.ins.sync_info
    if si is not None and si.on_wait and len(si.on_wait) > 1:
        waits = list(si.on_wait)
        si.on_wait = waits[:1]
        for w in waits[1:]:
            n = nc.sync.nop()
            n.ins.sync_info = bass_rust.SyncInfo(on_wait=[w], on_update=[])
    nc.all_engine_barrier()
    assert self.sems is not None
    popped = nc._tile_sem_poison_stack.pop()
    assert popped is self._sem_poison
    nc.clear_and_free_semaphores(list(self.sems.allocated().values()))
    nc.all_engine_barrier()


tile_mod.TileContext._drain_and_barrier = _patched_drain_and_barrier


# ------------------------------------------------------------- host consts
def _sine_pos(h, w, d):
    nf = d // 2
    scale = 2.0 * np.pi
    ye = np.arange(1, h + 1, dtype=np.float32) / h * scale
    xe = np.arange(1, w + 1, dtype=np.float32) / w * scale
    dim_t = 10000.0 ** (2.0 * (np.arange(nf) // 2).astype(np.float32) / nf)

    def enc(e):
        p = e[:, None] / dim_t
        return np.stack(
            [np.sin(p[:, 0::2]), np.cos(p[:, 1::2])], -1
        ).reshape(e.shape[0], nf)

    py, px = enc(ye), enc(xe)
    pos = np.concatenate(
        [
            np.broadcast_to(py[:, None, :], (h, w, nf)),
            np.broadcast_to(px[None, :, :], (h, w, nf)),
        ],
        -1,
    )
    return pos.reshape(h * w, d).astype(np.float32)


def _gind(F):
    per = F // GN_G
    ind = np.zeros((F, GN_G), np.float32)
    for g in range(GN_G):
        ind[g * per:(g + 1) * per, g] = 1.0
    return ind


def host_prep(inputs):
    f32 = lambda a: np.ascontiguousarray(np.asarray(a, np.float32))
    bf16 = lambda a: np.ascontiguousarray(
        np.asarray(a, np.float32).astype(NPBF))

    pos = _sine_pos(H, W, D)
    le = np.asarray(inputs["level_embed"], np.float32)
    posf = np.concatenate([pos + le[0], pos + le[1]], 0)        # [800, 512]
    gx = (np.arange(W, dtype=np.float32) + 0.5) / W
    gy = (np.arange(H, dtype=np.float32) + 0.5) / H
    X, Y = np.meshgrid(gx, gy)
    ref1 = np.stack([X, Y], -1).reshape(HW, 2)
    ref = np.concatenate([ref1, ref1], 0)                       # [800,2] (x,y)
    refg = np.empty((T, 2), np.float32)
    refg[:, 0] = ref[:, 0] * W - 0.5
    refg[:, 1] = ref[:, 1] * H - 0.5

    d = {}
    d["avT"] = bf16(np.asarray(inputs["av_w"]).T)
    d["aiT"] = bf16(np.asarray(inputs["ai_w"]).T)
    d["asT"] = bf16(np.asarray(inputs["as_w"]).T)
    d["convb_v"] = f32(np.asarray(inputs["av_b"])[:, None])
    d["convb_i"] = f32(np.asarray(inputs["ai_b"])[:, None])
    d["convb_s"] = f32(np.asarray(inputs["as_b"])[:, None])
    d["gng_v"] = f32(np.asarray(inputs["av_g"])[:, None])
    d["gnb_v"] = f32(np.asarray(inputs["av_be"])[:, None])
    d["gng_i"] = f32(np.asarray(inputs["ai_g"])[:, None])
    d["gnb_i"] = f32(np.asarray(inputs["ai_be"])[:, None])
    d["gng_s"] = f32(np.asarray(inputs["as_g"])[:, None])
    d["gnb_s"] = f32(np.asarray(inputs["as_be"])[:, None])
    d["ind512"] = f32(_gind(512))
    d["exp512"] = f32(_gind(512).T)
    d["ind768"] = f32(_gind(768))
    d["exp768"] = f32(_gind(768).T)
    d["ones128f"] = f32(np.ones((128, 1)))
    d["ones128b"] = bf16(np.ones((128, 1)))
    d["ones1f"] = f32(np.ones((1, 128)))
    d["ones1x128"] = bf16(np.ones((1, 128)))
    d["ident128"] = bf16(np.eye(128, dtype=np.float32))
    d["iota20"] = f32(np.broadcast_to(
        np.arange(20, dtype=np.float32)[None, :], (128, 20)))

    off_w = np.asarray(inputs["off_w"], np.float32)
    off_b = np.asarray(inputs["off_b"], np.float32)
    aw_w = np.asarray(inputs["aw_w"], np.float32)
    aw_b = np.asarray(inputs["aw_b"], np.float32)
    d["off_w"] = bf16(off_w)
    d["aw_w"] = bf16(aw_w)
    coord_add = np.einsum("td,ldj->ltj", posf, off_w) + off_b[:, None, :]
    ca = coord_add.reshape(NL, T, Hd * L * P, 2)
    ca[..., 0] += refg[None, :, None, 0]
    ca[..., 1] += refg[None, :, None, 1]
    d["coordAdd"] = f32(coord_add)
    d["awAdd"] = f32(np.einsum("td,ldj->ltj", posf, aw_w) + aw_b[:, None, :])
    d["vp_w"] = bf16(np.asarray(inputs["vp_w"]))
    d["vp_brow"] = bf16(np.asarray(inputs["vp_b"])[:, None, :])
    d["op_w"] = bf16(np.asarray(inputs["op_w"]))
    d["op_b"] = f32(np.asarray(inputs["op_b"])[:, :, None])
    d["ffn1_w"] = bf16(np.asarray(inputs["ffn1_w"]))
    d["ffn1_b"] = f32(np.asarray(inputs["ffn1_b"])[:, :, None])
    d["ffn2_w"] = bf16(np.asarray(inputs["ffn2_w"]))
    d["ffn2_b"] = f32(np.asarray(inputs["ffn2_b"])[:, :, None])
    for nm in ("ln1", "ln2"):
        g = np.asarray(inputs[f"{nm}_g"], np.float32)
        b = np.asarray(inputs[f"{nm}_b"], np.float32)
        d[nm] = f32(np.stack([g[:, 0], b[:, 0], g[:, 1], b[:, 1]], -1))
    return d


DRAM_SPECS = {
    "avT": ([C, D], BF16), "aiT": ([C, D], BF16), "asT": ([D, C], BF16),
    "convb_v": ([D, 1], F32), "convb_i": ([D, 1], F32),
    "convb_s": ([C, 1], F32),
    "gng_v": ([D, 1], F32), "gnb_v": ([D, 1], F32),
    "gng_i": ([D, 1], F32), "gnb_i": ([D, 1], F32),
    "gng_s": ([C, 1], F32), "gnb_s": ([C, 1], F32),
    "ind512": ([D, GN_G], F32), "exp512": ([GN_G, D], F32),
    "ind768": ([C, GN_G], F32), "exp768": ([GN_G, C], F32),
    "ones128f": ([128, 1], F32), "ones128b": ([128, 1], BF16),
    "ones1f": ([1, 128], F32),
    "ones1x128": ([1, 128], BF16),
    "ident128": ([128, 128], BF16),
    "iota20": ([128, 20], F32),
    "off_w": ([NL, D, 128], BF16), "aw_w": ([NL, D, 64], BF16),
    "coordAdd": ([NL, T, 128], F32), "awAdd": ([NL, T, 64], F32),
    "vp_w": ([NL, D, D], BF16), "vp_brow": ([NL, 1, D], BF16),
    "op_w": ([NL, D, D], BF16), "op_b": ([NL, D, 1], F32),
    "ffn1_w": ([NL, D, Dff], BF16), "ffn1_b": ([NL, Dff, 1], F32),
    "ffn2_w": ([NL, Dff, D], BF16), "ffn2_b": ([NL, D, 1], F32),
    "ln1": ([NL, D, 4], F32), "ln2": ([NL, D, 4], F32),
}

STREAMED = {"off_w", "aw_w", "coordAdd", "awAdd", "vp_w", "vp_brow", "op_w",
            "op_b", "ffn1_w", "ffn1_b", "ffn2_w", "ffn2_b", "ln1", "ln2",
            "avT", "aiT", "asT"}

# per-tag buffer counts for streamed layer weights (single-buffered)
W_BUFS = {"off_w": 4, "aw_w": 4, "vp_w": 4, "vp_brow": 1, "op_w": 4,
          "ffn1_w": 4, "ffn1_b": 8, "ffn2_w": 8, "ffn2_b": 4,
          "op_b": 4, "ln1": 4, "ln2": 4, "coordAdd": 7, "awAdd": 7}


def _split_multiwaits(nc, max_waits=1):
    """walrus rejects instructions carrying more than one sync-wait; hoist
    extra waits onto same-engine nops placed before the instruction."""
    for f in nc.m.functions:
        for blk in f.blocks:
            out = []
            changed = False
            for inst in blk.instructions:
                si = inst.sync_info
                if si is not None and si.on_wait and \
                        len(si.on_wait) > max_waits:
                    waits = list(si.on_wait)
                    for j, w in enumerate(waits[:-max_waits]):
                        n = mybir.InstNoOp(name=f"{inst.name}_w{j}", ins=[],
                                           outs=[])
                        n.engine = inst.engine
                        n.sync_info = bass_rust.SyncInfo(on_wait=[w],
                                                         on_update=[])
                        out.append(n)
                    si.on_wait = waits[-max_waits:]
                    changed = True
                out.append(inst)
            if changed:
                blk.instructions = out


class Ctx:
    pass


def build_kernel(num_layers=NL, num_batches=BPC, taps=()):
    nc = bass.Bass("TRN2", target_bir_lowering=False, debug=False,
                   num_devices=NCORES)
    g = Ctx()
    g.nc = nc
    g.NLs = num_layers
    g.BPCs = num_batches
    g.taps = set(taps)
    g.tap_d = {}

    g.dx_v = nc.dram_tensor("xv", [BPC, C, HW], F32, kind="ExternalInput")
    g.dx_i = nc.dram_tensor("xi", [BPC, C, HW], F32, kind="ExternalInput")
    g.out_d = nc.dram_tensor("out", [BPC, C, HW], F32, kind="ExternalOutput")
    g.dram = {nm: nc.dram_tensor(nm, shp, ty, kind="ExternalInput")
              for nm, (shp, ty) in DRAM_SPECS.items()}

    with TileContext(nc) as tc:
        g.tc = tc
        with contextlib.ExitStack() as ctx:
            _body(ctx, g)
    _split_multiwaits(nc)
    return nc


def _tap_fm(g, name, tiles, rows, cols, cast=True):
    """Dump feature-major tiles (list of [128, cols]) to a dram tap."""
    if name not in g.taps:
        return
    d = g.nc.dram_tensor(f"tap_{name}", [rows, cols], F32,
                         kind="ExternalOutput")
    g.tap_d[name] = d
    for k, tl in enumerate(tiles):
        r = min(128, rows - k * 128)
        if tl.dtype != F32 and cast:
            tf = g.scr.tile([128, cols], F32, tag="tapf")
            g.nc.vector.tensor_copy(tf[:r, :], tl[:r, :cols])
            g.dma(d[k * 128:k * 128 + r, :], tf[:r, :])
        else:
            g.dma(d[k * 128:k * 128 + r, :], tl[:r, :cols])


def _body(ctx, g):
    nc, tc = g.nc, g.tc
    pool = lambda name, bufs, **kw: ctx.enter_context(
        tc.tile_pool(name=name, bufs=bufs, **kw))
    g.cpool = pool("consts", 1)
    g.wpool = pool("weights", 1)
    g.state = pool("state", 1)
    g.scr = pool("scratch", 2)
    g.mps = pool("mpsum", 2, space="PSUM")
    g.sps = pool("spsum", 2, space="PSUM")
    g.dma = nc.sync.dma_start

    # ---------------- consts to SBUF ----------------
    cw = {}
    for nm, (shp, ty) in DRAM_SPECS.items():
        if nm in STREAMED:
            continue
        t = g.dram[nm]
        KX = shp[0]
        if KX <= 128:
            tl = g.cpool.tile([KX, shp[1]], ty, name=f"c_{nm}")
            g.dma(tl[:, :], t[:, :])
            cw[nm] = tl
        else:
            tiles = []
            for k0 in range(0, KX, 128):
                tl = g.cpool.tile([128, shp[1]], ty, name=f"c_{nm}_{k0}")
                g.dma(tl[:, :], t[k0:k0 + 128, :])
                tiles.append(tl)
            cw[nm] = tiles
    g.cw = cw

    # ---------------- persistent state ----------------
    g.srcF = [[g.state.tile([128, T], F32, name=f"srcF_{b}_{k}")
               for k in range(4)] for b in range(g.BPCs)]
    g.srcB = [[g.state.tile([128, T], BF16, name=f"srcB_{b}_{k}")
               for k in range(4)] for b in range(g.BPCs)]

    for b in range(g.BPCs):
        _input_stage(g, b)
    for l in range(g.NLs):
        lw = _load_layer_weights(g, l)
        for b in range(g.BPCs):
            _layer(g, l, b, lw)
    for b in range(g.BPCs):
        _output_stage(g, b)


def _psum_big(g):
    return g.mps.tile([128, 1024], F32, tag="big", name="psbig")


def _mm_acc(g, ps, pairs, n_total, m_rows=None, n_split=512):
    """ps[:mr, :n_total] = sum_k lhsT_k.T @ rhs_k  (bank-aligned N splits)."""
    mr = m_rows if m_rows is not None else ps.shape[0]
    for n0 in range(0, n_total, n_split):
        n1 = min(n0 + n_split, n_total)
        for i, (lt, rh) in enumerate(pairs):
            g.nc.tensor.matmul(
                ps[:mr, n0:n1], lt, rh[:, n0:n1],
                start=(i == 0), stop=(i == len(pairs) - 1),
            )


def _load_layer_weights(g, l):
    lw = {}

    def ld(name):
        t = g.dram[name]
        KX, MX = t.shape[1], t.shape[2]
        tiles = []
        for k0 in range(0, KX, 128):
            kk = min(128, KX - k0)
            tl = g.wpool.tile([128, MX], t.dtype, tag=f"w_{name}",
                              bufs=W_BUFS[name], name=f"{name}_l{l}_{k0}")
            g.dma(tl[:kk, :], t[l, k0:k0 + kk, :])
            tiles.append(tl)
        return tiles

    for nm in ("off_w", "aw_w", "vp_w", "vp_brow", "op_w", "ffn1_w",
               "ffn2_w", "op_b", "ffn1_b", "ffn2_b", "ln1", "ln2"):
        lw[nm] = ld(nm)
    lw["coordAdd"] = []
    lw["awAdd"] = []
    for (t0, sz) in TOKC:
        ca = g.wpool.tile([128, 128], F32, tag="w_coordAdd", bufs=7,
                          name=f"coordAdd_l{l}_{t0}")
        g.dma(ca[:sz, :], g.dram["coordAdd"][l, t0:t0 + sz, :])
        lw["coordAdd"].append(ca)
        aa = g.wpool.tile([128, 64], F32, tag="w_awAdd", bufs=7,
                          name=f"awAdd_l{l}_{t0}")
        g.dma(aa[:sz, :], g.dram["awAdd"][l, t0:t0 + sz, :])
        lw["awAdd"].append(aa)
    return lw


def _input_stage(g, b):
    nc, cw = g.nc, g.cw
    for (src_d, wT_d, bias, gg, gb) in (
        (g.dx_v, g.dram["avT"], cw["convb_v"], cw["gng_v"], cw["gnb_v"]),
        (g.dx_i, g.dram["aiT"], cw["convb_i"], cw["gng_i"], cw["gnb_i"]),
    ):
        half = 0 if src_d is g.dx_v else HW
        wT = []
        for k0 in range(0, C, 128):
            wt = g.scr.tile([128, D], BF16, tag="ffb", bufs=8, name="wTin")
            g.dma(wt[:, :], wT_d[k0:k0 + 128, :])
            wT.append(wt)
        xb = []
        for ki, k0 in enumerate(range(0, C, 128)):
            xf = g.scr.tile([128, HW], F32, tag="ln_sq", bufs=6, name="xf")
            g.dma(xf[:, :], src_d[b, k0:k0 + 128, :])
            xc = g.scr.tile([128, HW], BF16, tag="vtok", bufs=8, name="xc")
            nc.vector.tensor_copy(xc[:, :], xf[:, :])
            xb.append(xc)
        for m in range(4):
            ps = _psum_big(g)
            _mm_acc(g, ps, [(wT[k][:, m * 128:(m + 1) * 128], xb[k])
                            for k in range(6)], n_total=HW)
            nc.scalar.activation(g.srcF[b][m][:, half:half + HW],
                                 ps[:, :HW], AF.Identity,
                                 bias=bias[m][:, :],
                                 scale=1.0)
        _groupnorm(g, [g.srcF[b][k] for k in range(4)], half, HW,
                   cw["ind512"], cw["exp512"], 512, gg, gb,
                   out_bf=[(g.srcB[b][k], half) for k in range(4)])
    _tap_fm(g, f"src0_{b}", g.srcF[b], D, T, cast=False)


def _groupnorm(g, featF, col0, ncols, ind, exp, F, gcol, bcol, out_bf):
    """In-place f32 GroupNorm on feature-major tiles over columns
    [col0:col0+ncols]; optional bf16 shadow writes."""
    nc = g.nc
    nk = F // 128
    per = F // GN_G
    inv = 1.0 / (per * ncols)
    indl = ind if isinstance(ind, list) else [ind]

    ps = _psum_big(g)          # use [32, 2*ncols] view
    for k in range(nk):
        sq = g.scr.tile([128, ncols], F32, tag="gn_sq", bufs=2, name="gn_sq")
        nc.scalar.activation(sq[:, :], featF[k][:, col0:col0 + ncols],
                             AF.Square)
        it = indl[k] if len(indl) > 1 else indl[0]
        nc.tensor.matmul(ps[:GN_G, 0:ncols], it[:, :],
                         featF[k][:, col0:col0 + ncols],
                         start=(k == 0), stop=(k == nk - 1))
        nc.tensor.matmul(ps[:GN_G, 512:512 + ncols], it[:, :], sq[:, :],
                         start=(k == 0), stop=(k == nk - 1))
    red = g.scr.tile([GN_G, 2], F32, tag="gn_red", bufs=2)
    nc.vector.tensor_reduce(red[:, 0:1], ps[:GN_G, 0:ncols], AX.X, AL.add)
    nc.vector.tensor_reduce(red[:, 1:2], ps[:GN_G, 512:512 + ncols], AX.X,
                            AL.add)
    st = g.scr.tile([GN_G, 4], F32, tag="gn_st", bufs=2)
    # st0 = mean, st1 = E[x^2], st2 = var, st3 = rsqrt(var+eps)
    nc.vector.tensor_scalar(st[:, 0:2], red[:, 0:2], inv, None, AL.mult)
    nc.vector.tensor_tensor(st[:, 2:3], st[:, 0:1], st[:, 0:1], AL.mult)
    nc.vector.tensor_tensor(st[:, 2:3], st[:, 1:2], st[:, 2:3], AL.subtract)
    nc.vector.tensor_scalar(st[:, 2:3], st[:, 2:3], float(EPS), None, AL.add)
    nc.scalar.activation(st[:, 3:4], st[:, 2:3], AF.Sqrt)
    nc.vector.reciprocal(st[:, 3:4], st[:, 3:4])
    expl = exp if isinstance(exp, list) else [exp]
    for k in range(nk):
        et = (expl[0][:, k * 128:(k + 1) * 128] if len(expl) == 1
              else expl[k][:, :])
        eps_ = _psum_big(g)
        nc.tensor.matmul(eps_[:, 0:1], et, st[:, 0:1], start=True, stop=True)
        nc.tensor.matmul(eps_[:, 1:2], et, st[:, 3:4], start=True, stop=True)
        sc = g.scr.tile([128, 2], F32, tag="gn_sc", bufs=2)
        nc.vector.tensor_tensor(sc[:, 0:1], eps_[:, 1:2],
                                gcol[k][:, :], AL.mult)
        nc.vector.tensor_tensor(sc[:, 1:2], eps_[:, 0:1], sc[:, 0:1],
                                AL.mult)
        nc.vector.tensor_tensor(sc[:, 1:2], bcol[k][:, :],
                                sc[:, 1:2], AL.subtract)
        nc.vector.tensor_scalar(featF[k][:, col0:col0 + ncols],
                                featF[k][:, col0:col0 + ncols],
                                sc[:, 0:1], sc[:, 1:2], AL.mult, AL.add)
        if out_bf is not None:
            bt, boff = out_bf[k]
            nc.vector.tensor_copy(bt[:, boff:boff + ncols],
                                  featF[k][:, col0:col0 + ncols])


def _ln_spec(g, b, hF, lncols):
    """Modality-specific LayerNorm over features (partition axis), feature-
    major. hF: 4 f32 tiles [128, 800]. Stats come from bf16 shadows (exact
    f32 PSUM accumulation of bf16 inputs; rounding averages out over D).
    Writes srcF (f32) + srcB (bf16)."""
    nc, cw = g.nc, g.cw
    ones = cw["ones128b"]
    inv = 1.0 / D
    ps1 = _psum_big(g)         # row 0 = sum
    ps2 = _psum_big(g)         # row 0 = sumsq
    for k in range(4):
        hB = g.scr.tile([128, T], BF16, tag="ffb", bufs=8, name="ln_hb")
        nc.scalar.activation(hB[:, :], hF[k][:, :], AF.Copy)
        sq = g.scr.tile([128, T], BF16, tag="ffb", bufs=8, name="ln_sq2")
        nc.scalar.activation(sq[:, :], hB[:, :], AF.Square)
        for (n0, nn_) in NSPL:
            nc.tensor.matmul(ps1[0:1, n0:n0 + nn_], ones[:, :],
                             hB[:, n0:n0 + nn_],
                             start=(k == 0), stop=(k == 3))
            nc.tensor.matmul(ps2[0:1, n0:n0 + nn_], ones[:, :],
                             sq[:, n0:n0 + nn_],
                             start=(k == 0), stop=(k == 3))
    stm = g.scr.tile([1, T], F32, tag="ln_stm", bufs=2, name="ln_stm")
    sts = g.scr.tile([1, T], F32, tag="ln_sts", bufs=2, name="ln_sts")
    nc.vector.tensor_scalar(stm[:, :], ps1[0:1, 0:T], inv, None, AL.mult)
    nc.vector.tensor_scalar(sts[:, :], ps2[0:1, 0:T], inv, None, AL.mult)
    v = g.scr.tile([1, T], F32, tag="ln_v", bufs=2, name="ln_v")
    nc.vector.tensor_tensor(v[:, :], stm[:, :], stm[:, :], AL.mult)
    nc.vector.tensor_tensor(v[:, :], sts[:, :], v[:, :], AL.subtract)
    nc.vector.tensor_scalar(v[:, :], v[:, :], float(EPS), None, AL.add)
    nc.scalar.activation(sts[:, :], v[:, :], AF.Sqrt)
    nc.vector.reciprocal(sts[:, :], sts[:, :])
    # broadcast m/s rows to [128, T] via K=1 f32 matmuls (kept in PSUM)
    psm = _psum_big(g)
    pss = _psum_big(g)
    for psr, row in ((psm, stm), (pss, sts)):
        for (n0, nn_) in NSPL:
            nc.tensor.matmul(psr[:, n0:n0 + nn_], cw["ones1f"][:, :],
                             row[0:1, n0:n0 + nn_],
                             start=True, stop=True)
    for k in range(4):
        # in-place: h = (h - m) * s
        nc.vector.tensor_tensor(hF[k][:, :], hF[k][:, :], psm[:, 0:T],
                                AL.subtract)
        nc.vector.tensor_tensor(hF[k][:, :], hF[k][:, :], pss[:, 0:T],
                                AL.mult)
        lc = lncols[k]
        nc.scalar.activation(g.srcF[b][k][:, 0:HW], hF[k][:, 0:HW],
                             AF.Identity, bias=lc[:, 1:2], scale=lc[:, 0:1])
        nc.scalar.activation(g.srcF[b][k][:, HW:T], hF[k][:, HW:T],
                             AF.Identity, bias=lc[:, 3:4], scale=lc[:, 2:3])
        nc.vector.tensor_copy(g.srcB[b][k][:, :], g.srcF[b][k][:, :])


KCW = [128, 128, 128, 16]           # kv-chunk widths for kv=400


def _layer(g, l, b, lw):
    nc, cw = g.nc, g.cw
    srcB = g.srcB[b]

    # ---------------- value (token-major) -------------------------------
    vtok = []
    for i8 in range(8):
        lv, j = divmod(i8, 4)
        tbase = lv * HW + j * 128
        sz = min(128, (lv + 1) * HW - tbase)
        vt = g.scr.tile([128, D], BF16, tag="vtok", bufs=8,
                        name=f"vt_{l}_{b}_{i8}")
        ps = _psum_big(g)
        for (n0, nn_) in ((0, 512),):
            for k, w in enumerate(lw["vp_w"]):
                nc.tensor.matmul(ps[:sz, n0:n0 + nn_],
                                 srcB[k][:, tbase:tbase + sz],
                                 w[:, n0:n0 + nn_], start=(k == 0),
                                 stop=False)
            nc.tensor.matmul(ps[:sz, n0:n0 + nn_],
                             cw["ones1x128"][:, :sz],
                             lw["vp_brow"][0][0:1, n0:n0 + nn_],
                             start=False, stop=True)
        nc.scalar.activation(vt[:sz, :], ps[:sz, 0:D], AF.Copy)
        vtok.append(vt)

    # ------------------- coords + softmax (token-major) -----------------
    # coords ct [sz,128] f32: col 2g+0 = x_g, 2g+1 = y_g for group
    # g = (hp,hh,lv,p) lexicographic = 8*h + 4*lv + p.  awtB holds NEGATED
    # normalized attention weights (bf16).
    iota = cw["iota20"]
    coords, awtB = [], []
    for ci, (t0, sz) in enumerate(TOKC):
        ps = _psum_big(g)
        _mm_acc(g, ps, [(srcB[k][:, t0:t0 + sz], w)
                        for k, w in enumerate(lw["off_w"])],
                n_total=128, m_rows=sz)
        ct = g.scr.tile([128, 128], F32, tag="coords", bufs=7,
                        name=f"co_{l}_{b}_{ci}")
        nc.vector.tensor_tensor(ct[:sz, :], ps[:sz, 0:128],
                                lw["coordAdd"][ci][:sz, :], AL.add)
        coords.append(ct)

        ps2 = _psum_big(g)
        _mm_acc(g, ps2, [(srcB[k][:, t0:t0 + sz], w)
                         for k, w in enumerate(lw["aw_w"])],
                n_total=64, m_rows=sz)
        at = g.scr.tile([128, 64], F32, tag="awt", bufs=2,
                        name=f"aw_{l}_{b}_{ci}")
        nc.vector.tensor_tensor(at[:sz, :], ps2[:sz, 0:64],
                                lw["awAdd"][ci][:sz, :], AL.add)
        nc.scalar.activation(at[:sz, :], at[:sz, :], AF.Exp)
        at3 = at[:sz, :].rearrange("q (h e) -> q h e", e=8)
        sm = g.scr.tile([128, Hd], F32, tag="aw_sm", bufs=2)
        nc.vector.tensor_reduce(sm[:sz, :], at3, AX.X, AL.add)
        nc.vector.reciprocal(sm[:sz, :], sm[:sz, :])
        ab = g.scr.tile([128, 64], BF16, tag="awtB", bufs=7,
                        name=f"ab_{l}_{b}_{ci}")
        nc.vector.tensor_tensor(
            ab[:sz, :].rearrange("q (h e) -> q h e", e=8), at3,
            sm[:sz, :].unsqueeze(-1).broadcast_to([sz, Hd, 8]), AL.mult)
        awtB.append(ab)

    # ---- per head-pair block: tents, A build, PE transpose+P-sum, A@V --
    # Tents for the 16 (hh,lv,p) groups of the block:
    # sy[q, j*20+iy] = aw_g[q] * relu(1-|y_g[q]-iy|)   (aw folded, bf16)
    # tx[q, j*20+ix] =            relu(1-|x_g[q]-ix|)
    # via u = |d|-1 (<=0 on support): sy = min(u,0)*(-aw), tx = min(u,0)*(-1)
    attnT = [g.scr.tile([128, T], BF16, tag="attnT", bufs=4,
                        name=f"attnT_{l}_{b}_{k}") for k in range(4)]
    ident = cw["ident128"]
    sps_of = {}

    def emit_av(h, lv, atts):
        if h not in sps_of:
            sps_of[h] = g.sps.tile([64, T], F32, tag="samp", bufs=2,
                                   name="samp")
        sps = sps_of[h]
        for kc, att in enumerate(atts):
            kw = KCW[kc]
            vsl = vtok[lv * 4 + kc][:kw, h * Dh:(h + 1) * Dh]
            for (n0, nn_) in NSPL:
                nc.tensor.matmul(sps[:, n0:n0 + nn_], vsl,
                                 att[:kw, n0:n0 + nn_],
                                 start=(lv == 0 and kc == 0),
                                 stop=(lv == L - 1 and kc == 3))
        if lv == L - 1:
            kc_, ro = divmod(h * Dh, 128)
            nc.scalar.activation(attnT[kc_][ro:ro + Dh, :], sps[:, :],
                                 AF.Copy)
            del sps_of[h]

    pend = None
    for sb in range(Hd // 2):          # head-pair super-blocks
        syT, txT = [], []
        for ci, (t0, sz) in enumerate(TOKC):
            eng = nc.gpsimd if ci in (0, 1, 3, 5) else nc.vector
            dtag = "tdp" if ci in (0, 1, 3, 5) else "tdv"
            sy = g.scr.tile([128, 320], BF16, tag="sy", bufs=9,
                            name=f"sy_{l}_{b}_{sb}_{ci}")
            tx = g.scr.tile([128, 320], BF16, tag="tx", bufs=9,
                            name=f"tx_{l}_{b}_{sb}_{ci}")
            ct = coords[ci]
            for (dst, coff) in ((sy, 1), (tx, 0)):
                dd = g.scr.tile([128, 320], F32, tag=dtag, bufs=2,
                                name=f"dd_{l}_{b}_{sb}_{ci}_{coff}")
                d3 = dd[:sz, :].rearrange("q (g i) -> q g i", i=20)
                io = iota[:sz, :].unsqueeze(1).broadcast_to([sz, 16, 20])
                cc = ct[:sz, 32 * sb + coff:32 * (sb + 1):2] \
                    .unsqueeze(-1).broadcast_to([sz, 16, 20])
                eng.tensor_tensor(d3, io, cc, AL.subtract)
                nc.scalar.activation(dd[:sz, :], dd[:sz, :], AF.Abs)
                if coff == 1:
                    ty = g.scr.tile([128, 320], BF16, tag="tyr", bufs=2,
                                    name=f"ty_{l}_{b}_{sb}_{ci}")
                    nc.scalar.activation(ty[:sz, :], dd[:sz, :], AF.Relu,
                                         bias=1.0, scale=-1.0)
                    aw3 = awtB[ci][:sz, 16 * sb:16 * (sb + 1)] \
                        .unsqueeze(-1).broadcast_to([sz, 16, 20])
                    nc.vector.tensor_tensor(
                        dst[:sz, :].rearrange("q (g i) -> q g i", i=20),
                        ty[:sz, :].rearrange("q (g i) -> q g i", i=20),
                        aw3, AL.mult)
                else:
                    nc.scalar.activation(dst[:sz, :], dd[:sz, :], AF.Relu,
                                         bias=1.0, scale=-1.0)
            syT.append(sy)
            txT.append(tx)

        for hh in range(2):
            h = 2 * sb + hh
            for lv in range(L):
                g0 = (8 * hh + 4 * lv) * 20      # block-local group offset
                a4s = []
                for ci, (t0, sz) in enumerate(TOKC):
                    eng = nc.gpsimd if ci in (0, 1, 3, 5) else nc.vector
                    a4 = g.scr.tile([128, 1600], BF16, tag="a4", bufs=7,
                                    name=f"a4_{l}_{b}_{h}_{lv}_{ci}")
                    syv = syT[ci][:sz, g0:g0 + 80] \
                        .rearrange("q (p i) -> q p i", i=20) \
                        .unsqueeze(-1).broadcast_to([sz, 4, 20, 20])
                    txv = txT[ci][:sz, g0:g0 + 80] \
                        .rearrange("q (p x) -> q p x", x=20) \
                        .unsqueeze(2).broadcast_to([sz, 4, 20, 20])
                    eng.tensor_tensor(
                        a4[:sz, :].rearrange("q (p i x) -> q p i x",
                                             i=20, x=20),
                        syv, txv, AL.mult)
                    a4s.append(a4)
                atts = []
                for kc in range(4):
                    kw = KCW[kc]
                    psA = _psum_big(g)
                    for ci, (t0, sz) in enumerate(TOKC):
                        c0 = kc * 128
                        for p in range(P):
                            nc.tensor.matmul(
                                psA[:kw, t0:t0 + sz],
                                a4s[ci][:sz,
                                        p * 400 + c0:p * 400 + c0 + kw],
                                ident[:sz, :sz],
                                start=(p == 0), stop=(p == P - 1))
                    att = g.scr.tile([128, T], BF16, tag="atT", bufs=6,
                                     name=f"atT_{l}_{b}_{h}_{lv}_{kc}")
                    nc.scalar.activation(att[:kw, :], psA[:kw, 0:T],
                                         AF.Copy)
                    atts.append(att)
                if pend is not None:
                    emit_av(*pend)
                pend = (h, lv, atts)
    emit_av(*pend)

    _tap_fm(g, f"attnT_{l}_{b}", attnT, D, T)

    # ---------------- output projection + residual + LN1 ----------------
    hF = []
    for m in range(4):
        ps = _psum_big(g)
        _mm_acc(g, ps, [(w[:, m * 128:(m + 1) * 128], attnT[k])
                        for k, w in enumerate(lw["op_w"])], n_total=T)
        h_ = g.scr.tile([128, T], F32, tag="hF", bufs=4,
                        name=f"h1_{l}_{b}_{m}")
        nc.vector.scalar_tensor_tensor(
            h_[:, :], ps[:, 0:T], lw["op_b"][m][:, 0:1],
            g.srcF[b][m][:, :], AL.add, AL.add)
        hF.append(h_)
    _ln_spec(g, b, hF, lw["ln1"])
    _tap_fm(g, f"src1_{l}_{b}", g.srcF[b], D, T, cast=False)

    # ---------------- FFN + residual + LN2 ----------------
    ffb = []
    for m in range(8):
        ps = _psum_big(g)
        _mm_acc(g, ps, [(w[:, m * 128:(m + 1) * 128], g.srcB[b][k])
                        for k, w in enumerate(lw["ffn1_w"])], n_total=T)
        f_ = g.scr.tile([128, T], BF16, tag="ffb", bufs=8,
                        name=f"ff_{l}_{b}_{m}")
        nc.scalar.activation(f_[:, :], ps[:, 0:T], AF.Relu,
                             bias=lw["ffn1_b"][m][:, 0:1], scale=1.0)
        ffb.append(f_)
    hF2 = []
    for m in range(4):
        ps = _psum_big(g)
        _mm_acc(g, ps, [(w[:, m * 128:(m + 1) * 128], ffb[k])
                        for k, w in enumerate(lw["ffn2_w"])], n_total=T)
        h_ = g.scr.tile([128, T], F32, tag="hF", bufs=4,
                        name=f"h2_{l}_{b}_{m}")
        nc.vector.scalar_tensor_tensor(
            h_[:, :], ps[:, 0:T], lw["ffn2_b"][m][:, 0:1],
            g.srcF[b][m][:, :], AL.add, AL.add)
        hF2.append(h_)
    _ln_spec(g, b, hF2, lw["ln2"])
    _tap_fm(g, f"src2_{l}_{b}", g.srcF[b], D, T, cast=False)


def _output_stage(g, b):
    nc, cw = g.nc, g.cw
    asT = []
    for k0 in range(0, D, 128):
        wt = g.scr.tile([128, C], BF16, tag="ffb", bufs=8, name="wTout")
        g.dma(wt[:, :], g.dram["asT"][k0:k0 + 128, :])
        asT.append(wt)
    ob = []
    for k in range(4):
        s = g.scr.tile([128, HW], F32, tag="ln_sq", bufs=6, name="os")
        nc.vector.tensor_tensor(s[:, :], g.srcF[b][k][:, 0:HW],
                                g.srcF[b][k][:, HW:T], AL.add)
        sb = g.scr.tile([128, HW], BF16, tag="vtok", bufs=8, name="osb")
        nc.vector.tensor_copy(sb[:, :], s[:, :])
        ob.append(sb)
    outF = []
    for m in range(6):
        ps = _psum_big(g)
        _mm_acc(g, ps, [(asT[k][:, m * 128:(m + 1) * 128], ob[k])
                        for k in range(4)], n_total=HW)
        of = g.scr.tile([128, HW], F32, tag="ln_sq", bufs=6,
                        name=f"outF_{b}_{m}")
        nc.scalar.activation(of[:, :], ps[:, :HW], AF.Identity,
                             bias=cw["convb_s"][m][:, :],
                             scale=1.0)
        outF.append(of)
    _groupnorm(g, outF, 0, HW, cw["ind768"], cw["exp768"], C,
               cw["gng_s"], cw["gnb_s"], out_bf=None)
    for k in range(6):
        g.dma(g.out_d[b, k * 128:(k + 1) * 128, :], outF[k][:, :])


# ---------------------------------------------------------------- entry
_CACHED = {}


def _get_nc():
    if "nc" not in _CACHED:
        _CACHED["nc"] = build_kernel()
    return _CACHED["nc"]


def kernel(**inputs):
    consts = host_prep(inputs)
    xv = np.asarray(inputs["input_v"], np.float32).reshape(B, C, HW)
    xi = np.asarray(inputs["input_i"], np.float32).reshape(B, C, HW)
    nc = _get_nc()
    in_maps = []
    for c in range(NCORES):
        m = dict(consts)
        m["xv"] = np.ascontiguousarray(xv[c * BPC:(c + 1) * BPC])
        m["xi"] = np.ascontiguousarray(xi[c * BPC:(c + 1) * BPC])
        in_maps.append(m)
    res = run_bass_kernel_spmd(nc, in_maps, core_ids=list(range(NCORES)))
    out = np.concatenate([res.results[c]["out"] for c in range(NCORES)], 0)
    return out.reshape(B, C, H, W).astype(np.float32)



# revision 21
# speedup vs baseline: 1.6021x; 1.3130x over previous
"""Trainium2 Bass kernel for nn_Attention_Fusion_Bimodal_LNSpecific_Sum.

Bimodal deformable-attention encoder: conv1x1+GN on two modalities ->
concat tokens [800, 512] -> 6 encoder layers (deformable attention with
bilinear sampling, modality-specific LN, FFN) -> modality sum -> conv1x1+GN.

Sharding: pure data-parallel over batch (B=16 -> 2 batches per core x 8).

Device-side formulation highlights:
  - Bilinear sampling == dense sampling-matrix A[q, kv] @ V[kv, d] per
    (batch, head, level). A rows are separable tent products
    aw * relu(1-|y-iy|) * relu(1-|x-ix|) (exactly equivalent to the
    reference's corner gather + border clipping), built on the vector
    engine with stride-0 access-pattern expansion, then block-transposed
    via the DMA xbar (bf16) so TensorE can contract over kv.
  - Activations are feature-major [D, tok] for projections/LN; token-major
    tensors (coords, attention weights, value, A) come directly from matmul
    output orientation choices -- no layout transposes.
  - LN/GN partition-axis reductions run on TensorE via ones/indicator
    matmuls in f32 (PSUM accumulate), keeping norm stats at full precision.
"""

import contextlib

import numpy as np

import bass_rust
import concourse.bass as bass
import concourse.mybir as mybir
import concourse.tile as tile_mod
from bass_rust import ScopedClock
from concourse.tile import TileContext
from concourse.bass_utils import run_bass_kernel_spmd

dt = mybir.dt
AL = mybir.AluOpType
AF = mybir.ActivationFunctionType
AX = mybir.AxisListType

B, C, H, W = 16, 768, 20, 20
D, NL, Hd, L, P, Dff = 512, 6, 8, 2, 4, 1024
HW = H * W          # 400
T = 2 * HW          # 800
Dh = D // Hd        # 64
EPS = 1e-5
NCORES = 8
BPC = B // NCORES   # 2
GN_G = 32

F32 = dt.float32
BF16 = dt.bfloat16
NPBF = dt.np(BF16)

TOKC = [(0, 128), (128, 128), (256, 128), (384, 128),
        (512, 128), (640, 128), (768, 32)]
NSPL = [(0, 512), (512, 288)]       # psum-bank-aligned N splits for width 800


# ------------------------------------------------------- tile drain patch
def _patched_drain_and_barrier(self, tick_clock, wait_clock):
    nc = self.nc
    drain_inst = nc.sync.drain()
    wait_clock.add_sem_waits(
        drain_inst.ins, ScopedClock({None: tick_clock.global_clock})
    )
    si = drain_inst.ins.sync_info
    if si is not None and si.on_wait and len(si.on_wait) > 1:
        waits = list(si.on_wait)
        si.on_wait = waits[:1]
        for w in waits[1:]:
            n = nc.sync.nop()
            n.ins.sync_info = bass_rust.SyncInfo(on_wait=[w], on_update=[])
    nc.all_engine_barrier()
    assert self.sems is not None
    popped = nc._tile_sem_poison_stack.pop()
    assert popped is self._sem_poison
    nc.clear_and_free_semaphores(list(self.sems.allocated().values()))
    nc.all_engine_barrier()


tile_mod.TileContext._drain_and_barrier = _patched_drain_and_barrier


# ------------------------------------------------------------- host consts
def _sine_pos(h, w, d):
    nf = d // 2
    scale = 2.0 * np.pi
    ye = np.arange(1, h + 1, dtype=np.float32) / h * scale
    xe = np.arange(1, w + 1, dtype=np.float32) / w * scale
    dim_t = 10000.0 ** (2.0 * (np.arange(nf) // 2).astype(np.float32) / nf)

    def enc(e):
        p = e[:, None] / dim_t
        return np.stack(
            [np.sin(p[:, 0::2]), np.cos(p[:, 1::2])], -1
        ).reshape(e.shape[0], nf)

    py, px = enc(ye), enc(xe)
    pos = np.concatenate(
        [
            np.broadcast_to(py[:, None, :], (h, w, nf)),
            np.broadcast_to(px[None, :, :], (h, w, nf)),
        ],
        -1,
    )
    return pos.reshape(h * w, d).astype(np.float32)


def _gind(F):
    per = F // GN_G
    ind = np.zeros((F, GN_G), np.float32)
    for g in range(GN_G):
        ind[g * per:(g + 1) * per, g] = 1.0
    return ind


def host_prep(inputs):
    f32 = lambda a: np.ascontiguousarray(np.asarray(a, np.float32))
    bf16 = lambda a: np.ascontiguousarray(
        np.asarray(a, np.float32).astype(NPBF))

    pos = _sine_pos(H, W, D)
    le = np.asarray(inputs["level_embed"], np.float32)
    posf = np.concatenate([pos + le[0], pos + le[1]], 0)        # [800, 512]
    gx = (np.arange(W, dtype=np.float32) + 0.5) / W
    gy = (np.arange(H, dtype=np.float32) + 0.5) / H
    X, Y = np.meshgrid(gx, gy)
    ref1 = np.stack([X, Y], -1).reshape(HW, 2)
    ref = np.concatenate([ref1, ref1], 0)                       # [800,2] (x,y)
    refg = np.empty((T, 2), np.float32)
    refg[:, 0] = ref[:, 0] * W - 0.5
    refg[:, 1] = ref[:, 1] * H - 0.5

    d = {}
    d["avT"] = bf16(np.asarray(inputs["av_w"]).T)
    d["aiT"] = bf16(np.asarray(inputs["ai_w"]).T)
    d["asT"] = bf16(np.asarray(inputs["as_w"]).T)
    d["convb_v"] = f32(np.asarray(inputs["av_b"])[:, None])
    d["convb_i"] = f32(np.asarray(inputs["ai_b"])[:, None])
    d["convb_s"] = f32(np.asarray(inputs["as_b"])[:, None])
    d["gng_v"] = f32(np.asarray(inputs["av_g"])[:, None])
    d["gnb_v"] = f32(np.asarray(inputs["av_be"])[:, None])
    d["gng_i"] = f32(np.asarray(inputs["ai_g"])[:, None])
    d["gnb_i"] = f32(np.asarray(inputs["ai_be"])[:, None])
    d["gng_s"] = f32(np.asarray(inputs["as_g"])[:, None])
    d["gnb_s"] = f32(np.asarray(inputs["as_be"])[:, None])
    d["ind512"] = f32(_gind(512))
    d["exp512"] = f32(_gind(512).T)
    d["ind768"] = f32(_gind(768))
    d["exp768"] = f32(_gind(768).T)
    d["ones128f"] = f32(np.ones((128, 1)))
    d["ones128b"] = bf16(np.ones((128, 1)))
    d["ones1f"] = f32(np.ones((1, 128)))
    d["ones1x128"] = bf16(np.ones((1, 128)))
    d["ident128"] = bf16(np.eye(128, dtype=np.float32))
    d["iota20"] = f32(np.broadcast_to(
        np.arange(20, dtype=np.float32)[None, :], (128, 20)))

    off_w = np.asarray(inputs["off_w"], np.float32)
    off_b = np.asarray(inputs["off_b"], np.float32)
    aw_w = np.asarray(inputs["aw_w"], np.float32)
    aw_b = np.asarray(inputs["aw_b"], np.float32)
    d["off_w"] = bf16(off_w)
    d["aw_w"] = bf16(aw_w)
    coord_add = np.einsum("td,ldj->ltj", posf, off_w) + off_b[:, None, :]
    ca = coord_add.reshape(NL, T, Hd * L * P, 2)
    ca[..., 0] += refg[None, :, None, 0]
    ca[..., 1] += refg[None, :, None, 1]
    d["coordAdd"] = f32(coord_add)
    d["awAdd"] = f32(np.einsum("td,ldj->ltj", posf, aw_w) + aw_b[:, None, :])
    d["vp_w"] = bf16(np.asarray(inputs["vp_w"]))
    d["vp_brow"] = bf16(np.asarray(inputs["vp_b"])[:, None, :])
    d["op_w"] = bf16(np.asarray(inputs["op_w"]))
    d["op_b"] = f32(np.asarray(inputs["op_b"])[:, :, None])
    d["ffn1_w"] = bf16(np.asarray(inputs["ffn1_w"]))
    d["ffn1_b"] = f32(np.asarray(inputs["ffn1_b"])[:, :, None])
    d["ffn2_w"] = bf16(np.asarray(inputs["ffn2_w"]))
    d["ffn2_b"] = f32(np.asarray(inputs["ffn2_b"])[:, :, None])
    for nm in ("ln1", "ln2"):
        g = np.asarray(inputs[f"{nm}_g"], np.float32)
        b = np.asarray(inputs[f"{nm}_b"], np.float32)
        d[nm] = f32(np.stack([g[:, 0], b[:, 0], g[:, 1], b[:, 1]], -1))
    return d


DRAM_SPECS = {
    "avT": ([C, D], BF16), "aiT": ([C, D], BF16), "asT": ([D, C], BF16),
    "convb_v": ([D, 1], F32), "convb_i": ([D, 1], F32),
    "convb_s": ([C, 1], F32),
    "gng_v": ([D, 1], F32), "gnb_v": ([D, 1], F32),
    "gng_i": ([D, 1], F32), "gnb_i": ([D, 1], F32),
    "gng_s": ([C, 1], F32), "gnb_s": ([C, 1], F32),
    "ind512": ([D, GN_G], F32), "exp512": ([GN_G, D], F32),
    "ind768": ([C, GN_G], F32), "exp768": ([GN_G, C], F32),
    "ones128f": ([128, 1], F32), "ones128b": ([128, 1], BF16),
    "ones1f": ([1, 128], F32),
    "ones1x128": ([1, 128], BF16),
    "ident128": ([128, 128], BF16),
    "iota20": ([128, 20], F32),
    "off_w": ([NL, D, 128], BF16), "aw_w": ([NL, D, 64], BF16),
    "coordAdd": ([NL, T, 128], F32), "awAdd": ([NL, T, 64], F32),
    "vp_w": ([NL, D, D], BF16), "vp_brow": ([NL, 1, D], BF16),
    "op_w": ([NL, D, D], BF16), "op_b": ([NL, D, 1], F32),
    "ffn1_w": ([NL, D, Dff], BF16), "ffn1_b": ([NL, Dff, 1], F32),
    "ffn2_w": ([NL, Dff, D], BF16), "ffn2_b": ([NL, D, 1], F32),
    "ln1": ([NL, D, 4], F32), "ln2": ([NL, D, 4], F32),
}

STREAMED = {"off_w", "aw_w", "coordAdd", "awAdd", "vp_w", "vp_brow", "op_w",
            "op_b", "ffn1_w", "ffn1_b", "ffn2_w", "ffn2_b", "ln1", "ln2",
            "avT", "aiT", "asT"}

# per-tag buffer counts for streamed layer weights (single-buffered)
W_BUFS = {"off_w": 4, "aw_w": 4, "vp_w": 4, "vp_brow": 1, "op_w": 4,
          "ffn1_w": 4, "ffn1_b": 8, "ffn2_w": 8, "ffn2_b": 4,
          "op_b": 4, "ln1": 4, "ln2": 4, "coordAdd": 7, "awAdd": 7}


def _split_multiwaits(nc, max_waits=1):
    """walrus rejects instructions carrying more than one sync-wait; hoist
    extra waits onto same-engine nops placed before the instruction."""
    for f in nc.m.functions:
        for blk in f.blocks:
            out = []
            changed = False
            for inst in blk.instructions:
                si = inst.sync_info
                if si is not None and si.on_wait and \
                        len(si.on_wait) > max_waits:
                    waits = list(si.on_wait)
                    for j, w in enumerate(waits[:-max_waits]):
                        n = mybir.InstNoOp(name=f"{inst.name}_w{j}", ins=[],
                                           outs=[])
                        n.engine = inst.engine
                        n.sync_info = bass_rust.SyncInfo(on_wait=[w],
                                                         on_update=[])
                        out.append(n)
                    si.on_wait = waits[-max_waits:]
                    changed = True
                out.append(inst)
            if changed:
                blk.instructions = out


class Ctx:
    pass


def build_kernel(num_layers=NL, num_batches=BPC, taps=()):
    nc = bass.Bass("TRN2", target_bir_lowering=False, debug=False,
                   num_devices=NCORES)
    g = Ctx()
    g.nc = nc
    g.NLs = num_layers
    g.BPCs = num_batches
    g.taps = set(taps)
    g.tap_d = {}

    g.dx_v = nc.dram_tensor("xv", [BPC, C, HW], F32, kind="ExternalInput")
    g.dx_i = nc.dram_tensor("xi", [BPC, C, HW], F32, kind="ExternalInput")
    g.out_d = nc.dram_tensor("out", [BPC, C, HW], F32, kind="ExternalOutput")
    g.dram = {nm: nc.dram_tensor(nm, shp, ty, kind="ExternalInput")
              for nm, (shp, ty) in DRAM_SPECS.items()}

    with TileContext(nc) as tc:
        g.tc = tc
        with contextlib.ExitStack() as ctx:
            _body(ctx, g)
    _split_multiwaits(nc)
    return nc


def _tap_fm(g, name, tiles, rows, cols, cast=True):
    """Dump feature-major tiles (list of [128, cols]) to a dram tap."""
    if name not in g.taps:
        return
    d = g.nc.dram_tensor(f"tap_{name}", [rows, cols], F32,
                         kind="ExternalOutput")
    g.tap_d[name] = d
    for k, tl in enumerate(tiles):
        r = min(128, rows - k * 128)
        if tl.dtype != F32 and cast:
            tf = g.scr.tile([128, cols], F32, tag="tapf")
            g.nc.vector.tensor_copy(tf[:r, :], tl[:r, :cols])
            g.dma(d[k * 128:k * 128 + r, :], tf[:r, :])
        else:
            g.dma(d[k * 128:k * 128 + r, :], tl[:r, :cols])


def _body(ctx, g):
    nc, tc = g.nc, g.tc
    pool = lambda name, bufs, **kw: ctx.enter_context(
        tc.tile_pool(name=name, bufs=bufs, **kw))
    g.cpool = pool("consts", 1)
    g.wpool = pool("weights", 1)
    g.state = pool("state", 1)
    g.scr = pool("scratch", 2)
    g.mps = pool("mpsum", 2, space="PSUM")
    g.sps = pool("spsum", 2, space="PSUM")
    g.dma = nc.sync.dma_start

    # ---------------- consts to SBUF ----------------
    cw = {}
    for nm, (shp, ty) in DRAM_SPECS.items():
        if nm in STREAMED:
            continue
        t = g.dram[nm]
        KX = shp[0]
        if KX <= 128:
            tl = g.cpool.tile([KX, shp[1]], ty, name=f"c_{nm}")
            g.dma(tl[:, :], t[:, :])
            cw[nm] = tl
        else:
            tiles = []
            for k0 in range(0, KX, 128):
                tl = g.cpool.tile([128, shp[1]], ty, name=f"c_{nm}_{k0}")
                g.dma(tl[:, :], t[k0:k0 + 128, :])
                tiles.append(tl)
            cw[nm] = tiles
    g.cw = cw

    # ---------------- persistent state ----------------
    g.srcF = [[g.state.tile([128, T], F32, name=f"srcF_{b}_{k}")
               for k in range(4)] for b in range(g.BPCs)]
    g.srcB = [[g.state.tile([128, T], BF16, name=f"srcB_{b}_{k}")
               for k in range(4)] for b in range(g.BPCs)]

    for b in range(g.BPCs):
        _input_stage(g, b)
    for l in range(g.NLs):
        lw = _load_layer_weights(g, l)
        for b in range(g.BPCs):
            _layer(g, l, b, lw)
    for b in range(g.BPCs):
        _output_stage(g, b)


def _psum_big(g):
    return g.mps.tile([128, 1024], F32, tag="big", name="psbig")


def _mm_acc(g, ps, pairs, n_total, m_rows=None, n_split=512):
    """ps[:mr, :n_total] = sum_k lhsT_k.T @ rhs_k  (bank-aligned N splits)."""
    mr = m_rows if m_rows is not None else ps.shape[0]
    for n0 in range(0, n_total, n_split):
        n1 = min(n0 + n_split, n_total)
        for i, (lt, rh) in enumerate(pairs):
            g.nc.tensor.matmul(
                ps[:mr, n0:n1], lt, rh[:, n0:n1],
                start=(i == 0), stop=(i == len(pairs) - 1),
            )


def _load_layer_weights(g, l):
    lw = {}

    def ld(name):
        t = g.dram[name]
        KX, MX = t.shape[1], t.shape[2]
        tiles = []
        for k0 in range(0, KX, 128):
            kk = min(128, KX - k0)
            tl = g.wpool.tile([128, MX], t.dtype, tag=f"w_{name}",
                              bufs=W_BUFS[name], name=f"{name}_l{l}_{k0}")
            g.dma(tl[:kk, :], t[l, k0:k0 + kk, :])
            tiles.append(tl)
        return tiles

    for nm in ("off_w", "aw_w", "vp_w", "vp_brow", "op_w", "ffn1_w",
               "ffn2_w", "op_b", "ffn1_b", "ffn2_b", "ln1", "ln2"):
        lw[nm] = ld(nm)
    lw["coordAdd"] = []
    lw["awAdd"] = []
    for (t0, sz) in TOKC:
        ca = g.wpool.tile([128, 128], F32, tag="w_coordAdd", bufs=7,
                          name=f"coordAdd_l{l}_{t0}")
        g.dma(ca[:sz, :], g.dram["coordAdd"][l, t0:t0 + sz, :])
        lw["coordAdd"].append(ca)
        aa = g.wpool.tile([128, 64], F32, tag="w_awAdd", bufs=7,
                          name=f"awAdd_l{l}_{t0}")
        g.dma(aa[:sz, :], g.dram["awAdd"][l, t0:t0 + sz, :])
        lw["awAdd"].append(aa)
    return lw


def _input_stage(g, b):
    nc, cw = g.nc, g.cw
    for (src_d, wT_d, bias, gg, gb) in (
        (g.dx_v, g.dram["avT"], cw["convb_v"], cw["gng_v"], cw["gnb_v"]),
        (g.dx_i, g.dram["aiT"], cw["convb_i"], cw["gng_i"], cw["gnb_i"]),
    ):
        half = 0 if src_d is g.dx_v else HW
        wT = []
        for k0 in range(0, C, 128):
            wt = g.scr.tile([128, D], BF16, tag="ffb", bufs=8, name="wTin")
            g.dma(wt[:, :], wT_d[k0:k0 + 128, :])
            wT.append(wt)
        xb = []
        for ki, k0 in enumerate(range(0, C, 128)):
            xf = g.scr.tile([128, HW], F32, tag="ln_sq", bufs=6, name="xf")
            g.dma(xf[:, :], src_d[b, k0:k0 + 128, :])
            xc = g.scr.tile([128, HW], BF16, tag="vtok", bufs=8, name="xc")
            nc.vector.tensor_copy(xc[:, :], xf[:, :])
            xb.append(xc)
        for m in range(4):
            ps = _psum_big(g)
            _mm_acc(g, ps, [(wT[k][:, m * 128:(m + 1) * 128], xb[k])
                            for k in range(6)], n_total=HW)
            nc.scalar.activation(g.srcF[b][m][:, half:half + HW],
                                 ps[:, :HW], AF.Identity,
                                 bias=bias[m][:, :],
                                 scale=1.0)
        _groupnorm(g, [g.srcF[b][k] for k in range(4)], half, HW,
                   cw["ind512"], cw["exp512"], 512, gg, gb,
                   out_bf=[(g.srcB[b][k], half) for k in range(4)])
    _tap_fm(g, f"src0_{b}", g.srcF[b], D, T, cast=False)


def _groupnorm(g, featF, col0, ncols, ind, exp, F, gcol, bcol, out_bf):
    """In-place f32 GroupNorm on feature-major tiles over columns
    [col0:col0+ncols]; optional bf16 shadow writes."""
    nc = g.nc
    nk = F // 128
    per = F // GN_G
    inv = 1.0 / (per * ncols)
    indl = ind if isinstance(ind, list) else [ind]

    ps = _psum_big(g)          # use [32, 2*ncols] view
    for k in range(nk):
        sq = g.scr.tile([128, ncols], F32, tag="gn_sq", bufs=2, name="gn_sq")
        nc.scalar.activation(sq[:, :], featF[k][:, col0:col0 + ncols],
                             AF.Square)
        it = indl[k] if len(indl) > 1 else indl[0]
        nc.tensor.matmul(ps[:GN_G, 0:ncols], it[:, :],
                         featF[k][:, col0:col0 + ncols],
                         start=(k == 0), stop=(k == nk - 1))
        nc.tensor.matmul(ps[:GN_G, 512:512 + ncols], it[:, :], sq[:, :],
                         start=(k == 0), stop=(k == nk - 1))
    red = g.scr.tile([GN_G, 2], F32, tag="gn_red", bufs=2)
    nc.vector.tensor_reduce(red[:, 0:1], ps[:GN_G, 0:ncols], AX.X, AL.add)
    nc.vector.tensor_reduce(red[:, 1:2], ps[:GN_G, 512:512 + ncols], AX.X,
                            AL.add)
    st = g.scr.tile([GN_G, 4], F32, tag="gn_st", bufs=2)
    # st0 = mean, st1 = E[x^2], st2 = var, st3 = rsqrt(var+eps)
    nc.vector.tensor_scalar(st[:, 0:2], red[:, 0:2], inv, None, AL.mult)
    nc.vector.tensor_tensor(st[:, 2:3], st[:, 0:1], st[:, 0:1], AL.mult)
    nc.vector.tensor_tensor(st[:, 2:3], st[:, 1:2], st[:, 2:3], AL.subtract)
    nc.vector.tensor_scalar(st[:, 2:3], st[:, 2:3], float(EPS), None, AL.add)
    nc.scalar.activation(st[:, 3:4], st[:, 2:3], AF.Sqrt)
    nc.vector.reciprocal(st[:, 3:4], st[:, 3:4])
    expl = exp if isinstance(exp, list) else [exp]
    for k in range(nk):
        et = (expl[0][:, k * 128:(k + 1) * 128] if len(expl) == 1
              else expl[k][:, :])
        eps_ = _psum_big(g)
        nc.tensor.matmul(eps_[:, 0:1], et, st[:, 0:1], start=True, stop=True)
        nc.tensor.matmul(eps_[:, 1:2], et, st[:, 3:4], start=True, stop=True)
        sc = g.scr.tile([128, 2], F32, tag="gn_sc", bufs=2)
        nc.vector.tensor_tensor(sc[:, 0:1], eps_[:, 1:2],
                                gcol[k][:, :], AL.mult)
        nc.vector.tensor_tensor(sc[:, 1:2], eps_[:, 0:1], sc[:, 0:1],
                                AL.mult)
        nc.vector.tensor_tensor(sc[:, 1:2], bcol[k][:, :],
                                sc[:, 1:2], AL.subtract)
        nc.vector.tensor_scalar(featF[k][:, col0:col0 + ncols],
                                featF[k][:, col0:col0 + ncols],
                                sc[:, 0:1], sc[:, 1:2], AL.mult, AL.add)
        if out_bf is not None:
            bt, boff = out_bf[k]
            nc.vector.tensor_copy(bt[:, boff:boff + ncols],
                                  featF[k][:, col0:col0 + ncols])


def _ln_spec(g, b, hF, lncols):
    """Modality-specific LayerNorm over features (partition axis), feature-
    major. hF: 4 f32 tiles [128, 800]. Stats come from bf16 shadows (exact
    f32 PSUM accumulation of bf16 inputs; rounding averages out over D).
    Writes srcF (f32) + srcB (bf16)."""
    nc, cw = g.nc, g.cw
    ones = cw["ones128b"]
    inv = 1.0 / D
    ps1 = _psum_big(g)         # row 0 = sum
    ps2 = _psum_big(g)         # row 0 = sumsq
    for k in range(4):
        hB = g.scr.tile([128, T], BF16, tag="ffb", bufs=8, name="ln_hb")
        nc.scalar.activation(hB[:, :], hF[k][:, :], AF.Copy)
        sq = g.scr.tile([128, T], BF16, tag="ffb", bufs=8, name="ln_sq2")
        nc.scalar.activation(sq[:, :], hB[:, :], AF.Square)
        for (n0, nn_) in NSPL:
            nc.tensor.matmul(ps1[0:1, n0:n0 + nn_], ones[:, :],
                             hB[:, n0:n0 + nn_],
                             start=(k == 0), stop=(k == 3))
            nc.tensor.matmul(ps2[0:1, n0:n0 + nn_], ones[:, :],
                             sq[:, n0:n0 + nn_],
                             start=(k == 0), stop=(k == 3))
    stm = g.scr.tile([1, T], F32, tag="ln_stm", bufs=2, name="ln_stm")
    sts = g.scr.tile([1, T], F32, tag="ln_sts", bufs=2, name="ln_sts")
    nc.vector.tensor_scalar(stm[:, :], ps1[0:1, 0:T], inv, None, AL.mult)
    nc.vector.tensor_scalar(sts[:, :], ps2[0:1, 0:T], inv, None, AL.mult)
    v = g.scr.tile([1, T], F32, tag="ln_v", bufs=2, name="ln_v")
    nc.vector.tensor_tensor(v[:, :], stm[:, :], stm[:, :], AL.mult)
    nc.vector.tensor_tensor(v[:, :], sts[:, :], v[:, :], AL.subtract)
    nc.vector.tensor_scalar(v[:, :], v[:, :], float(EPS), None, AL.add)
    nc.scalar.activation(sts[:, :], v[:, :], AF.Sqrt)
    nc.vector.reciprocal(sts[:, :], sts[:, :])
    # broadcast m/s rows to [128, T] via K=1 f32 matmuls (kept in PSUM)
    psm = _psum_big(g)
    pss = _psum_big(g)
    for psr, row in ((psm, stm), (pss, sts)):
        for (n0, nn_) in NSPL:
            nc.tensor.matmul(psr[:, n0:n0 + nn_], cw["ones1f"][:, :],
                             row[0:1, n0:n0 + nn_],
                             start=True, stop=True)
    for k in range(4):
        # in-place: h = (h - m) * s
        nc.vector.tensor_tensor(hF[k][:, :], hF[k][:, :], psm[:, 0:T],
                                AL.subtract)
        nc.vector.tensor_tensor(hF[k][:, :], hF[k][:, :], pss[:, 0:T],
                                AL.mult)
        lc = lncols[k]
        nc.scalar.activation(g.srcF[b][k][:, 0:HW], hF[k][:, 0:HW],
                             AF.Identity, bias=lc[:, 1:2], scale=lc[:, 0:1])
        nc.scalar.activation(g.srcF[b][k][:, HW:T], hF[k][:, HW:T],
                             AF.Identity, bias=lc[:, 3:4], scale=lc[:, 2:3])
        nc.vector.tensor_copy(g.srcB[b][k][:, :], g.srcF[b][k][:, :])


KCW = [128, 128, 128, 16]           # kv-chunk widths for kv=400


def _layer(g, l, b, lw):
    nc, cw = g.nc, g.cw
    srcB = g.srcB[b]

    # ---------------- value (token-major) -------------------------------
    vtok = []
    for i8 in range(8):
        lv, j = divmod(i8, 4)
        tbase = lv * HW + j * 128
        sz = min(128, (lv + 1) * HW - tbase)
        vt = g.scr.tile([128, D], BF16, tag="vtok", bufs=8,
                        name=f"vt_{l}_{b}_{i8}")
        ps = _psum_big(g)
        for (n0, nn_) in ((0, 512),):
            for k, w in enumerate(lw["vp_w"]):
                nc.tensor.matmul(ps[:sz, n0:n0 + nn_],
                                 srcB[k][:, tbase:tbase + sz],
                                 w[:, n0:n0 + nn_], start=(k == 0),
                                 stop=False)
            nc.tensor.matmul(ps[:sz, n0:n0 + nn_],
                             cw["ones1x128"][:, :sz],
                             lw["vp_brow"][0][0:1, n0:n0 + nn_],
                             start=False, stop=True)
        nc.scalar.activation(vt[:sz, :], ps[:sz, 0:D], AF.Copy)
        vtok.append(vt)

    # ------------------- coords + softmax (token-major) -----------------
    # coords ct [sz,128] f32: col 2g+0 = x_g, 2g+1 = y_g for group
    # g = (hp,hh,lv,p) lexicographic = 8*h + 4*lv + p.  awtB holds NEGATED
    # normalized attention weights (bf16).
    iota = cw["iota20"]
    coords, awtB = [], []
    for ci, (t0, sz) in enumerate(TOKC):
        ps = _psum_big(g)
        _mm_acc(g, ps, [(srcB[k][:, t0:t0 + sz], w)
                        for k, w in enumerate(lw["off_w"])],
                n_total=128, m_rows=sz)
        ct = g.scr.tile([128, 128], F32, tag="coords", bufs=7,
                        name=f"co_{l}_{b}_{ci}")
        nc.vector.tensor_tensor(ct[:sz, :], ps[:sz, 0:128],
                                lw["coordAdd"][ci][:sz, :], AL.add)
        coords.append(ct)

        ps2 = _psum_big(g)
        _mm_acc(g, ps2, [(srcB[k][:, t0:t0 + sz], w)
                         for k, w in enumerate(lw["aw_w"])],
                n_total=64, m_rows=sz)
        at = g.scr.tile([128, 64], F32, tag="awt", bufs=2,
                        name=f"aw_{l}_{b}_{ci}")
        nc.vector.tensor_tensor(at[:sz, :], ps2[:sz, 0:64],
                                lw["awAdd"][ci][:sz, :], AL.add)
        nc.scalar.activation(at[:sz, :], at[:sz, :], AF.Exp)
        at3 = at[:sz, :].rearrange("q (h e) -> q h e", e=8)
        sm = g.scr.tile([128, Hd], F32, tag="aw_sm", bufs=2)
        nc.vector.tensor_reduce(sm[:sz, :], at3, AX.X, AL.add)
        nc.vector.reciprocal(sm[:sz, :], sm[:sz, :])
        ab = g.scr.tile([128, 64], BF16, tag="awtB", bufs=7,
                        name=f"ab_{l}_{b}_{ci}")
        nc.vector.tensor_tensor(
            ab[:sz, :].rearrange("q (h e) -> q h e", e=8), at3,
            sm[:sz, :].unsqueeze(-1).broadcast_to([sz, Hd, 8]), AL.mult)
        awtB.append(ab)

    # ---- per head-pair block: tents, A build, PE transpose+P-sum, A@V --
    # Tents for the 16 (hh,lv,p) groups of the block:
    # sy[q, j*20+iy] = aw_g[q] * relu(1-|y_g[q]-iy|)   (aw folded, bf16)
    # tx[q, j*20+ix] =            relu(1-|x_g[q]-ix|)
    # via u = |d|-1 (<=0 on support): sy = min(u,0)*(-aw), tx = min(u,0)*(-1)
    attnT = [g.scr.tile([128, T], BF16, tag="attnT", bufs=4,
                        name=f"attnT_{l}_{b}_{k}") for k in range(4)]
    ident = cw["ident128"]
    sps_of = {}

    def emit_av(h, lv, atts):
        if h not in sps_of:
            sps_of[h] = g.sps.tile([64, T], F32, tag="samp", bufs=2,
                                   name="samp")
        sps = sps_of[h]
        for kc, att in enumerate(atts):
            kw = KCW[kc]
            vsl = vtok[lv * 4 + kc][:kw, h * Dh:(h + 1) * Dh]
            for (n0, nn_) in NSPL:
                nc.tensor.matmul(sps[:, n0:n0 + nn_], vsl,
                                 att[:kw, n0:n0 + nn_],
                                 start=(lv == 0 and kc == 0),
                                 stop=(lv == L - 1 and kc == 3))
        if lv == L - 1:
            kc_, ro = divmod(h * Dh, 128)
            nc.scalar.activation(attnT[kc_][ro:ro + Dh, :], sps[:, :],
                                 AF.Copy)
            del sps_of[h]

    pend = None
    for sb in range(Hd // 2):          # head-pair super-blocks
        syT, txT = [], []
        for ci, (t0, sz) in enumerate(TOKC):
            eng = nc.gpsimd if ci in (1, 3) else nc.vector
            dtag = "tdp" if ci in (1, 3) else "tdv"
            sy = g.scr.tile([128, 320], BF16, tag="sy", bufs=9,
                            name=f"sy_{l}_{b}_{sb}_{ci}")
            tx = g.scr.tile([128, 320], BF16, tag="tx", bufs=9,
                            name=f"tx_{l}_{b}_{sb}_{ci}")
            ct = coords[ci]
            for (dst, coff) in ((sy, 1), (tx, 0)):
                dd = g.scr.tile([128, 320], F32, tag=dtag, bufs=2,
                                name=f"dd_{l}_{b}_{sb}_{ci}_{coff}")
                d3 = dd[:sz, :].rearrange("q (g i) -> q g i", i=20)
                io = iota[:sz, :].unsqueeze(1).broadcast_to([sz, 16, 20])
                cc = ct[:sz, 32 * sb + coff:32 * (sb + 1):2] \
                    .unsqueeze(-1).broadcast_to([sz, 16, 20])
                eng.tensor_tensor(d3, io, cc, AL.subtract)
                nc.scalar.activation(dd[:sz, :], dd[:sz, :], AF.Abs)
                if coff == 1:
                    ty = g.scr.tile([128, 320], BF16, tag="tyr", bufs=2,
                                    name=f"ty_{l}_{b}_{sb}_{ci}")
                    nc.scalar.activation(ty[:sz, :], dd[:sz, :], AF.Relu,
                                         bias=1.0, scale=-1.0)
                    aw3 = awtB[ci][:sz, 16 * sb:16 * (sb + 1)] \
                        .unsqueeze(-1).broadcast_to([sz, 16, 20])
                    nc.vector.tensor_tensor(
                        dst[:sz, :].rearrange("q (g i) -> q g i", i=20),
                        ty[:sz, :].rearrange("q (g i) -> q g i", i=20),
                        aw3, AL.mult)
                else:
                    nc.scalar.activation(dst[:sz, :], dd[:sz, :], AF.Relu,
                                         bias=1.0, scale=-1.0)
            syT.append(sy)
            txT.append(tx)

        for hh in range(2):
            h = 2 * sb + hh
            for lv in range(L):
                g0 = (8 * hh + 4 * lv) * 20      # block-local group offset
                a4s = []
                for ci, (t0, sz) in enumerate(TOKC):
                    eng = nc.gpsimd if ci in (1, 3) else nc.vector
                    a4 = g.scr.tile([128, 1600], BF16, tag="a4", bufs=7,
                                    name=f"a4_{l}_{b}_{h}_{lv}_{ci}")
                    syv = syT[ci][:sz, g0:g0 + 80] \
                        .rearrange("q (p i) -> q p i", i=20) \
                        .unsqueeze(-1).broadcast_to([sz, 4, 20, 20])
                    txv = txT[ci][:sz, g0:g0 + 80] \
                        .rearrange("q (p x) -> q p x", x=20) \
                        .unsqueeze(2).broadcast_to([sz, 4, 20, 20])
                    eng.tensor_tensor(
                        a4[:sz, :].rearrange("q (p i x) -> q p i x",
                                             i=20, x=20),
                        syv, txv, AL.mult)
                    a4s.append(a4)
                atts = []
                for kc in range(4):
                    kw = KCW[kc]
                    psA = _psum_big(g)
                    for ci, (t0, sz) in enumerate(TOKC):
                        c0 = kc * 128
                        for p in range(P):
                            nc.tensor.matmul(
                                psA[:kw, t0:t0 + sz],
                                a4s[ci][:sz,
                                        p * 400 + c0:p * 400 + c0 + kw],
                                ident[:sz, :sz],
                                start=(p == 0), stop=(p == P - 1))
                    att = g.scr.tile([128, T], BF16, tag="atT", bufs=6,
                                     name=f"atT_{l}_{b}_{h}_{lv}_{kc}")
                    nc.scalar.activation(att[:kw, :], psA[:kw, 0:T],
                                         AF.Copy)
                    atts.append(att)
                if pend is not None:
                    emit_av(*pend)
                pend = (h, lv, atts)
    emit_av(*pend)

    _tap_fm(g, f"attnT_{l}_{b}", attnT, D, T)

    # ---------------- output projection + residual + LN1 ----------------
    hF = []
    for m in range(4):
        ps = _psum_big(g)
        _mm_acc(g, ps, [(w[:, m * 128:(m + 1) * 128], attnT[k])
                        for k, w in enumerate(lw["op_w"])], n_total=T)
        h_ = g.scr.tile([128, T], F32, tag="hF", bufs=4,
                        name=f"h1_{l}_{b}_{m}")
        nc.vector.scalar_tensor_tensor(
            h_[:, :], ps[:, 0:T], lw["op_b"][m][:, 0:1],
            g.srcF[b][m][:, :], AL.add, AL.add)
        hF.append(h_)
    _ln_spec(g, b, hF, lw["ln1"])
    _tap_fm(g, f"src1_{l}_{b}", g.srcF[b], D, T, cast=False)

    # ---------------- FFN + residual + LN2 ----------------
    ffb = []
    for m in range(8):
        ps = _psum_big(g)
        _mm_acc(g, ps, [(w[:, m * 128:(m + 1) * 128], g.srcB[b][k])
                        for k, w in enumerate(lw["ffn1_w"])], n_total=T)
        f_ = g.scr.tile([128, T], BF16, tag="ffb", bufs=8,
                        name=f"ff_{l}_{b}_{m}")
        nc.scalar.activation(f_[:, :], ps[:, 0:T], AF.Relu,
                             bias=lw["ffn1_b"][m][:, 0:1], scale=1.0)
        ffb.append(f_)
    hF2 = []
    for m in range(4):
        ps = _psum_big(g)
        _mm_acc(g, ps, [(w[:, m * 128:(m + 1) * 128], ffb[k])
                        for k, w in enumerate(lw["ffn2_w"])], n_total=T)
        h_ = g.scr.tile([128, T], F32, tag="hF", bufs=4,
                        name=f"h2_{l}_{b}_{m}")
        nc.vector.scalar_tensor_tensor(
            h_[:, :], ps[:, 0:T], lw["ffn2_b"][m][:, 0:1],
            g.srcF[b][m][:, :], AL.add, AL.add)
        hF2.append(h_)
    _ln_spec(g, b, hF2, lw["ln2"])
    _tap_fm(g, f"src2_{l}_{b}", g.srcF[b], D, T, cast=False)


def _output_stage(g, b):
    nc, cw = g.nc, g.cw
    asT = []
    for k0 in range(0, D, 128):
        wt = g.scr.tile([128, C], BF16, tag="ffb", bufs=8, name="wTout")
        g.dma(wt[:, :], g.dram["asT"][k0:k0 + 128, :])
        asT.append(wt)
    ob = []
    for k in range(4):
        s = g.scr.tile([128, HW], F32, tag="ln_sq", bufs=6, name="os")
        nc.vector.tensor_tensor(s[:, :], g.srcF[b][k][:, 0:HW],
                                g.srcF[b][k][:, HW:T], AL.add)
        sb = g.scr.tile([128, HW], BF16, tag="vtok", bufs=8, name="osb")
        nc.vector.tensor_copy(sb[:, :], s[:, :])
        ob.append(sb)
    outF = []
    for m in range(6):
        ps = _psum_big(g)
        _mm_acc(g, ps, [(asT[k][:, m * 128:(m + 1) * 128], ob[k])
                        for k in range(4)], n_total=HW)
        of = g.scr.tile([128, HW], F32, tag="ln_sq", bufs=6,
                        name=f"outF_{b}_{m}")
        nc.scalar.activation(of[:, :], ps[:, :HW], AF.Identity,
                             bias=cw["convb_s"][m][:, :],
                             scale=1.0)
        outF.append(of)
    _groupnorm(g, outF, 0, HW, cw["ind768"], cw["exp768"], C,
               cw["gng_s"], cw["gnb_s"], out_bf=None)
    for k in range(6):
        g.dma(g.out_d[b, k * 128:(k + 1) * 128, :], outF[k][:, :])


# ---------------------------------------------------------------- entry
_CACHED = {}


def _get_nc():
    if "nc" not in _CACHED:
        _CACHED["nc"] = build_kernel()
    return _CACHED["nc"]


def kernel(**inputs):
    consts = host_prep(inputs)
    xv = np.asarray(inputs["input_v"], np.float32).reshape(B, C, HW)
    xi = np.asarray(inputs["input_i"], np.float32).reshape(B, C, HW)
    nc = _get_nc()
    in_maps = []
    for c in range(NCORES):
        m = dict(consts)
        m["xv"] = np.ascontiguousarray(xv[c * BPC:(c + 1) * BPC])
        m["xi"] = np.ascontiguousarray(xi[c * BPC:(c + 1) * BPC])
        in_maps.append(m)
    res = run_bass_kernel_spmd(nc, in_maps, core_ids=list(range(NCORES)))
    out = np.concatenate([res.results[c]["out"] for c in range(NCORES)], 0)
    return out.reshape(B, C, H, W).astype(np.float32)

